# revision 1
# baseline (speedup 1.0000x reference)
"""Trainium2 Bass kernel for the CSSAM sparse-attention module.

Math (per batch b):
  q_in  = src[b] viewed as [C, L] (L = 64*64 = 4096)               (queries)
  kv[j, l] = featpad[b, j//9, kh + 2*oh - 1, kw + 2*ow - 1]
             where (kh, kw) = divmod(j % 9, 3), l = oh*64 + ow     (keys/vals)
      -> only feat channels 0..28 are ever used (first 256 of C*9 unfold rows)
  Q^T = Wq @ q_in + bq ; K^T = Wk @ kv + bk ; V = (Wk-like conv)   [C, L]
  per head h (8 heads, d = 32): softmax((Qh^T)^T Kh / sqrt(d)) Vh
  out[b] = (Wo @ O^T + (Wo bv + bo)) * src[b]

Sharding: 8 cores = 2 batches x 4 query-chunks of 1024. K/V work is
replicated across the 4 cores of a batch; everything stays on-device.
K^T and V are computed directly from feat as a 9-tap stride-2 convolution
(matmul accumulation over kernel taps with strided SBUF access patterns),
so the im2col "unfold" is never materialized.

Softmax uses no max-subtraction (scores are tiny: |s| < 1 by construction
of the module: w_scale=0.02 projections of unit-normal data).
Denominators come from ones-vector matmuls accumulated alongside PV^T;
the 1/denom row is broadcast back to 32 partitions with a K=1 matmul.
"""

from contextlib import ExitStack

import numpy as np

import concourse.bass as bass
import concourse.mybir as mybir
import concourse.tile as tile

F32 = mybir.dt.float32
F32R = mybir.dt.float32r
BF16 = mybir.dt.bfloat16
AF = mybir.ActivationFunctionType
ALU = mybir.AluOpType

B = 2
C = 256
NH = 8
HD = 32
H = W = 64
L = H * W            # 4096 query / kv positions per batch
HF = WF = 128        # feat spatial
CF = 29              # feat channels actually used by the module
FR = WF + 2          # zero-padded frame width
NCORE = 8
QCHUNK = L // 4      # 1024 queries per core
QN = 256             # attention q sub-chunk (PSUM-bank friendly)
NQC = QCHUNK // QN   # 4
KT = L // 128        # 32 key tiles
SCALE = float(1.0 / np.sqrt(HD))


def build_kernel(nc: bass.Bass):
    featc = nc.declare_dram_parameter("featc", [CF, HF, WF], BF16, isOutput=False)
    srcq = nc.declare_dram_parameter("srcq", [C, QCHUNK], F32, isOutput=False)
    wqt = nc.declare_dram_parameter("wqt", [128, 2, C], F32, isOutput=False)
    wot = nc.declare_dram_parameter("wot", [128, 2, C], F32, isOutput=False)
    wkc = nc.declare_dram_parameter("wkc", [32, 9, C], BF16, isOutput=False)
    wvc = nc.declare_dram_parameter("wvc", [32, 9, C], BF16, isOutput=False)
    bq2 = nc.declare_dram_parameter("bq2", [128, 2], F32, isOutput=False)
    bk2 = nc.declare_dram_parameter("bk2", [128, 2], F32, isOutput=False)
    boe = nc.declare_dram_parameter("boe", [128, 2], F32, isOutput=False)
    onesd = nc.declare_dram_parameter("onesd", [128, 32], BF16, isOutput=False)
    outq = nc.declare_dram_parameter("outq", [C, QCHUNK], F32, isOutput=True)

    with ExitStack() as ctx:
        ctx.enter_context(
            nc.allow_low_precision("float32r tiles carry full fp32 bits")
        )
        tc = ctx.enter_context(tile.TileContext(nc))
        const = ctx.enter_context(tc.tile_pool(name="const", bufs=1))
        convp = ctx.enter_context(tc.tile_pool(name="convp", bufs=1))
        work = ctx.enter_context(tc.tile_pool(name="work", bufs=2))
        pwork = ctx.enter_context(tc.tile_pool(name="pwork", bufs=4))
        psc = ctx.enter_context(tc.tile_pool(name="psc", bufs=2, space="PSUM"))
        pacc = ctx.enter_context(tc.tile_pool(name="pacc", bufs=2, space="PSUM"))

        # ---- constant / input loads ----
        wqt_sb = const.tile([128, 2, C], F32R, tag="wqt")
        nc.sync.dma_start(wqt_sb[:], wqt[:].bitcast(F32R))
        wot_sb = const.tile([128, 2, C], F32R, tag="wot")
        nc.sync.dma_start(wot_sb[:], wot[:].bitcast(F32R))
        wkc_sb = convp.tile([32, 9, C], BF16, tag="wkc")
        nc.sync.dma_start(wkc_sb[:], wkc[:])
        wvc_sb = convp.tile([32, 9, C], BF16, tag="wvc")
        nc.sync.dma_start(wvc_sb[:], wvc[:])
        bq2_sb = const.tile([128, 2], F32, tag="bq2")
        nc.sync.dma_start(bq2_sb[:], bq2[:])
        bk2_sb = const.tile([128, 2], F32, tag="bk2")
        nc.sync.dma_start(bk2_sb[:], bk2[:])
        boe_sb = const.tile([128, 2], F32, tag="boe")
        nc.sync.dma_start(boe_sb[:], boe[:])
        srcq_sb = const.tile([128, 2, QCHUNK], F32R, tag="srcq")
        nc.sync.dma_start(srcq_sb[:], srcq.rearrange("(o p) n -> p o n", p=128).bitcast(F32R))
        srcf_sb = const.tile([128, 2, QCHUNK], F32, tag="srcf")
        nc.sync.dma_start(srcf_sb[:], srcq.rearrange("(o p) n -> p o n", p=128))
        ones_sb = const.tile([128, 32], BF16, tag="ones")
        nc.sync.dma_start(ones_sb[:], onesd[:])

        # feat with a baked zero border (only row 0 / col 0 are ever read
        # out-of-bounds: kh=0,oh=0 and kw=0,ow=0)
        feat_sb = convp.tile([32, FR * FR], BF16, tag="feat")
        feat3 = feat_sb.rearrange("p (r c) -> p r c", c=FR)
        nc.vector.memset(feat3[0:CF, 0:1, :], 0.0)
        nc.vector.memset(feat3[0:CF, :, 0:1], 0.0)
        nc.sync.dma_start(feat3[0:CF, 1 : HF + 1, 1 : WF + 1], featc[:])

        # ---- Q^T = Wq @ src_chunk + bq   -> [C(part, 2 tiles), QCHUNK] ----
        qT_sb = const.tile([128, 2, QCHUNK], BF16, tag="qT")
        for jo in range(2):
            for qn in range(2):
                ps = psc.tile([128, 4 * QN], F32, tag="sc", name=f"q_ps{jo}{qn}")
                ps = ps[:, 0:512]
                for ki in range(2):
                    nc.tensor.matmul(
                        ps[:],
                        (wqt_sb[:, ki, jo * 128 : (jo + 1) * 128]),
                        (srcq_sb[:, ki, qn * 512 : (qn + 1) * 512]),
                        start=(ki == 0),
                        stop=(ki == 1),
                    )
                nc.vector.tensor_scalar_add(
                    qT_sb[:, jo, qn * 512 : (qn + 1) * 512], ps[:], bq2_sb[:, jo : jo + 1]
                )

        # ---- K^T: 9-tap stride-2 conv over feat -> [C(part, 2 tiles), L] ----
        kT_sb = const.tile([128, 2, L], BF16, tag="kT")
        for jo in range(2):
            for ln in range(8):
                ps = psc.tile([128, 4 * QN], F32, tag="sc", name=f"k_ps{jo}{ln}")
                ps = ps[:, 0:512]
                oh0 = ln * 8
                for kk in range(9):
                    kh, kw = divmod(kk, 3)
                    rhs = feat3[
                        0:CF,
                        kh + 2 * oh0 : kh + 2 * oh0 + 16 : 2,
                        kw : kw + 2 * W : 2,
                    ]
                    nc.tensor.matmul(
                        ps[:],
                        (wkc_sb[0:CF, kk, jo * 128 : (jo + 1) * 128]),
                        (rhs),
                        start=(kk == 0),
                        stop=(kk == 8),
                    )
                nc.vector.tensor_scalar_add(
                    kT_sb[:, jo, ln * 512 : (ln + 1) * 512], ps[:], bk2_sb[:, jo : jo + 1]
                )

        # ---- V: same conv, transposed orientation -> [l(part, 32 tiles), C] ----
        v_sb = const.tile([128, KT, C], BF16, tag="v")
        for lt in range(KT):
            ps = psc.tile([128, 4 * QN], F32, tag="sc", name=f"v_ps{lt}")
            for half in range(2):
                oh = 2 * lt + half
                for kk in range(9):
                    kh, kw = divmod(kk, 3)
                    lhsT = feat3[0:CF, kh + 2 * oh, kw : kw + 2 * W : 2]
                    nc.tensor.matmul(
                        ps[64 * half : 64 * half + 64, 0:C],
                        (lhsT),
                        (wvc_sb[0:CF, kk, :]),
                        start=(kk == 0),
                        stop=(kk == 8),
                        tile_position=(0, 64 * half),
                        skip_group_check=True,
                    )
            nc.vector.tensor_copy(v_sb[:, lt, :], ps[:, 0:C])

        # ---- attention over 4 q sub-chunks of 256 ----
        for qc in range(NQC):
            u_ps = [
                pacc.tile([128, 512], F32, tag="uacc", name=f"u{qc}_{i}")[:, 0:QN]
                for i in range(2)
            ]
            d_ps = [
                pacc.tile([128, 512], F32, tag="dacc", name=f"d{qc}_{i}")[:, 0:QN]
                for i in range(2)
            ]
            for kt in range(KT):
                # scores tile t holds row-groups g=2t, 2t+1: bank b <-> one
                # row group (both jo halves share the row slot, so the PE
                # serializes same-bank writes; distinct groups hit distinct
                # banks and run concurrently)
                p_tiles = []
                for t in range(2):
                    sc = psc.tile([128, 4 * QN], F32, tag="sc", name=f"sc{qc}_{kt}_{t}")
                    for g in (2 * t, 2 * t + 1):
                        for jo in range(2):
                            col = (2 * (g % 2) + jo) * QN
                            nc.tensor.matmul(
                                sc[:, col : col + QN],
                                (kT_sb[32 * g : 32 * g + 32, jo, kt * 128 : (kt + 1) * 128]),
                                (qT_sb[32 * g : 32 * g + 32, jo, qc * QN : (qc + 1) * QN]),
                                start=True,
                                stop=True,
                                tile_position=(32 * g, 0),
                                skip_group_check=True,
                            )
                    p_sb = pwork.tile([128, 4 * QN], BF16, tag="p", name=f"p{qc}_{kt}_{t}")
                    nc.scalar.activation(p_sb[:], sc[:], AF.Exp, scale=SCALE)
                    p_tiles.append(p_sb)
                for h in range(NH):
                    g, jo = h % 4, h // 4
                    psl = p_tiles[g // 2][:, (2 * (g % 2) + jo) * QN :][:, 0:QN]
                    nc.tensor.matmul(
                        u_ps[jo][32 * g : 32 * g + 32, :],
                        (v_sb[:, kt, 32 * h : 32 * h + 32]),
                        psl,
                        start=(kt == 0),
                        stop=(kt == KT - 1),
                        tile_position=(0, 32 * g),
                        skip_group_check=True,
                    )
                    nc.tensor.matmul(
                        d_ps[jo][32 * g : 32 * g + 1, :],
                        (ones_sb[:, 0:1]),
                        psl,
                        start=(kt == 0),
                        stop=(kt == KT - 1),
                        tile_position=(0, 32 * g),
                        skip_group_check=True,
                    )

            # normalize: rec = 1/denom rows, broadcast to 32 partitions via
            # K=1 diagonal-packed matmuls, then O^T = U * rec_bcast
            rec_sb = work.tile([128, 2 * QN], F32, tag="rec")
            for jo in range(2):
                for g in range(4):
                    nc.vector.reciprocal(
                        rec_sb[32 * g : 32 * g + 1, jo * QN : (jo + 1) * QN],
                        d_ps[jo][32 * g : 32 * g + 1, :],
                    )
            # split 1/denom into bf16 hi + residual, broadcast to 32
            # partitions with two accumulating diag-packed bf16 matmuls
            rec_hi = work.tile([128, 2 * QN], BF16, tag="rec_hi")
            rec_lo = work.tile([128, 2 * QN], BF16, tag="rec_lo")
            for jo in range(2):
                for g in range(4):
                    r = slice(32 * g, 32 * g + 1)
                    q = slice(jo * QN, (jo + 1) * QN)
                    nc.vector.tensor_copy(rec_hi[r, q], rec_sb[r, q])
                    nc.vector.tensor_sub(rec_lo[r, q], rec_sb[r, q], rec_hi[r, q])
            rb = psc.tile([128, 4 * QN], F32, tag="sc", name=f"rb{qc}")
            for jo in range(2):
                for g in range(4):
                    for part, st in ((rec_hi, True), (rec_lo, False)):
                        nc.tensor.matmul(
                            rb[32 * g : 32 * g + 32, jo * QN : (jo + 1) * QN],
                            ones_sb[32 * g : 32 * g + 1, 0:32],
                            part[32 * g : 32 * g + 1, jo * QN : (jo + 1) * QN],
                            start=st,
                            stop=not st,
                            tile_position=(32 * g, 32 * g),
                            skip_group_check=True,
                        )
            rb_sb = work.tile([128, 2 * QN], F32, tag="rb")
            nc.vector.tensor_copy(rb_sb[:], rb[:, 0 : 2 * QN])
            o_sb = work.tile([128, 2, QN], F32R, tag="o")
            for jo in range(2):
                nc.vector.tensor_tensor(
                    o_sb[:, jo, :],
                    u_ps[jo][:, :],
                    rb_sb[:, jo * QN : (jo + 1) * QN],
                    ALU.mult,
                )

            # out-projection + bias + * src, then store
            for jo in range(2):
                op = pacc.tile([128, 512], F32, tag="dacc", name=f"op{qc}_{jo}")[:, 0:QN]
                for ki in range(2):
                    nc.tensor.matmul(
                        op[:],
                        (wot_sb[:, ki, jo * 128 : (jo + 1) * 128]),
                        (o_sb[:, ki, :]),
                        start=(ki == 0),
                        stop=(ki == 1),
                    )
                ot = work.tile([128, QN], F32, tag="ot")
                nc.vector.tensor_scalar_add(ot[:], op[:], boe_sb[:, jo : jo + 1])
                nc.vector.tensor_tensor(
                    ot[:],
                    ot[:],
                    srcf_sb[:, jo, qc * QN : (qc + 1) * QN],
                    ALU.mult,
                )
                nc.sync.dma_start(
                    outq[jo * 128 : (jo + 1) * 128, qc * QN : (qc + 1) * QN], ot[:]
                )

    return nc


_CACHE: dict = {}


def _split_matmul_waits(nc: bass.Bass):
    """walrus's fp32r self-loading matmul (S3 LW struct) accepts only one
    sync-wait command; peel extra waits onto PE EventSemaphore ops inserted
    immediately before the matmul (same sync point, so no deadlock risk)."""
    import bass_rust

    n_new = 0
    for fn in nc.m.functions:
        for block in fn.blocks:
            insts = list(block.instructions)
            out = []
            changed = False
            skip = (
                mybir.InstEventSemaphore,
                mybir.InstAllEngineBarrier,
                mybir.InstHalt,
            )
            for inst in insts:
                if not isinstance(inst, skip) and inst.sync_info is not None:
                    si = inst.sync_info
                    waits = list(si.on_wait)
                    if len(waits) > 1:
                        for w in waits[:-1]:
                            ev = mybir.InstEventSemaphore(
                                name=f"WSPLIT-{n_new}", ins=[], outs=[]
                            )
                            ev.engine = inst.engine
                            ev.sync_info = bass_rust.SyncInfo(
                                on_wait=[w], on_update=[]
                            )
                            out.append(ev)
                            n_new += 1
                        inst.sync_info = bass_rust.SyncInfo(
                            on_wait=[waits[-1]], on_update=list(si.on_update)
                        )
                        changed = True
                out.append(inst)
            if changed:
                block.instructions = out
    return n_new


def get_nc() -> bass.Bass:
    if "nc" not in _CACHE:
        nc = bass.Bass()
        build_kernel(nc)
        _split_matmul_waits(nc)
        nc.finalize()
        _CACHE["nc"] = nc
    return _CACHE["nc"]


def make_core_inputs(feat, src, Wq, bq, Wk, bk, Wv, bv, Wo, bo):
    """Host-side sharding / layout prep. Returns list of 8 input dicts."""
    f32 = np.float32
    feat = np.asarray(feat, f32)
    src = np.asarray(src, f32)
    Wq, Wk, Wv, Wo = (np.asarray(x, f32) for x in (Wq, Wk, Wv, Wo))
    bq, bk, bv, bo = (np.asarray(x, f32) for x in (bq, bk, bv, bo))

    wqt = np.ascontiguousarray(Wq.T.reshape(2, 128, C).transpose(1, 0, 2))
    wot = np.ascontiguousarray(Wo.T.reshape(2, 128, C).transpose(1, 0, 2))

    # conv-tap layouts: wkc[cp, kk, cout] = Wk[cout, 9*cp + kk] (0 beyond C)
    import ml_dtypes

    bf16 = ml_dtypes.bfloat16
    wkc = np.zeros((32, 9, C), f32)
    wvc = np.zeros((32, 9, C), f32)
    cp_idx, kk_idx = np.meshgrid(np.arange(CF), np.arange(9), indexing="ij")
    j = (9 * cp_idx + kk_idx).ravel()
    valid = j < C
    wkc[cp_idx.ravel()[valid], kk_idx.ravel()[valid], :] = Wk[:, j[valid]].T
    wvc[cp_idx.ravel()[valid], kk_idx.ravel()[valid], :] = Wv[:, j[valid]].T
    wkc = wkc.astype(bf16)
    wvc = wvc.astype(bf16)
    onesd = np.ones((128, 32), bf16)

    bq2 = np.ascontiguousarray(bq.reshape(2, 128).T)
    bk2 = np.ascontiguousarray(bk.reshape(2, 128).T)
    boev = Wo @ bv + bo
    boe = np.ascontiguousarray(boev.reshape(2, 128).T)

    shared = dict(
        wqt=wqt, wot=wot, wkc=wkc, wvc=wvc, bq2=bq2, bk2=bk2, boe=boe, onesd=onesd
    )
    in_maps = []
    for core in range(NCORE):
        b, qi = divmod(core, 4)
        m = dict(shared)
        m["featc"] = np.ascontiguousarray(feat[b, :CF]).astype(bf16)
        m["srcq"] = np.ascontiguousarray(
            src[b].reshape(C, L)[:, qi * QCHUNK : (qi + 1) * QCHUNK]
        )
        in_maps.append(m)
    return in_maps


def _ensure_ntff_hook():
    """Provide antenv.axon_hooks if the image lacks it (needed for trace=True).

    Mirrors trn_agent_boot.trn_boot._ntff_profile_via_ctypes: drives NTFF
    profiling via the axon PJRT .so's C ABI.
    """
    import contextlib
    import ctypes
    import os
    import sys
    import types

    try:
        import antenv.axon_hooks  # noqa: F401

        return
    except ImportError:
        pass

    mod = types.ModuleType("antenv.axon_hooks")
    box = [None]
    mod.set_axon_ntff_profile_hook = lambda h: box.__setitem__(0, h)
    mod.get_axon_ntff_profile_hook = lambda: box[0]
    sys.modules["antenv.axon_hooks"] = mod
    import antenv

    antenv.axon_hooks = mod

    so_path = os.environ.get("PJRT_LIBRARY_PATH", "/opt/axon/libaxon_pjrt.so")
    try:
        lib = ctypes.CDLL(so_path)
    except OSError:
        return
    if not hasattr(lib, "axon_start_nrt_profile"):
        return
    lib.axon_start_nrt_profile.argtypes = [
        ctypes.POINTER(ctypes.c_int64),
        ctypes.c_size_t,
    ]
    lib.axon_start_nrt_profile.restype = ctypes.c_int64
    lib.axon_stop_nrt_profile.argtypes = [ctypes.c_char_p]
    lib.axon_stop_nrt_profile.restype = ctypes.c_int64

    @contextlib.contextmanager
    def _hook(output_dir, device_ids):
        import jax

        jax.devices()
        if device_ids:
            ids = (ctypes.c_int64 * len(device_ids))(*device_ids)
            rc = lib.axon_start_nrt_profile(ids, len(device_ids))
        else:
            rc = lib.axon_start_nrt_profile(None, 0)
        if rc != 0:
            raise RuntimeError(f"axon_start_nrt_profile rc={rc}")
        try:
            yield
        finally:
            n = lib.axon_stop_nrt_profile(str(output_dir).encode())
            print(f"profile: {n} file(s) written to {output_dir}", file=sys.stderr)

    box[0] = _hook


def run(inputs: dict, trace: bool = False, trace_cores=None):
    _ensure_ntff_hook()
    from concourse.bass_utils import run_bass_kernel_spmd

    nc = get_nc()
    in_maps = make_core_inputs(**inputs)
    res = run_bass_kernel_spmd(
        nc,
        in_maps,
        list(range(NCORE)),
        trace=trace,
        trace_cores=trace_cores,
    )
    out = np.empty((B, C, L), np.float32)
    for core in range(NCORE):
        b, qi = divmod(core, 4)
        out[b, :, qi * QCHUNK : (qi + 1) * QCHUNK] = res.results[core]["outq"]
    return out.reshape(B, C, H, W), res


def kernel(feat, src, Wq, bq, Wk, bk, Wv, bv, Wo, bo):
    out, _ = run(
        dict(feat=feat, src=src, Wq=Wq, bq=bq, Wk=Wk, bk=bk, Wv=Wv, bv=bv, Wo=Wo, bo=bo)
    )
    return out



# revision 15
# speedup vs baseline: 2.1410x; 2.1410x over previous
"""Trainium2 Bass kernel for the CSSAM sparse-attention module (v2).

Math (per batch b):
  q_in  = src[b] viewed as [C, L] (L = 64*64 = 4096)               (queries)
  kv[j, l] = featpad[b, j//9, kh + 2*oh - 1, kw + 2*ow - 1]
             where (kh, kw) = divmod(j % 9, 3), l = oh*64 + ow     (keys/vals)
      -> only feat channels 0..28 are ever used
  Q^T = Wq @ q_in + bq ; K^T = Wk @ kv + bk ; V = kv^T Wv^T
  per head h (8 heads, d = 32): softmax((Qh^T)^T Kh / sqrt(d)) Vh
  out[b] = (Wo @ O^T + (Wo bv + bo)) * src[b]

Sharding: 8 cores = 2 batches x 4 query-chunks of 1024; K/V work is
replicated across the 4 cores of a batch.

v2 design notes (HW-model driven):
 - The kernel is Activation-engine bound: exp over 8 heads x 4096 kv x
   1024 q per core = 262144 rows/partition ~ 218 us busy minimum. All
   other engines are paced to hide under it.
 - Scores matmuls use 4-way row-group concurrency (tile_position=(32g,0));
   PV + denominator use 4-way col-group concurrency (tile_position=(0,32g)).
 - Denominator rows are broadcast to all 32 partitions of a group by a
   [128, 32] ones lhsT (M=32), so normalization is a plain elementwise
   multiply by 1/D. D accumulates in its own PSUM bank - interleaving the
   D accumulation group into the U bank corrupts U on hardware.
 - K/V convs pack (channel, kh) into 87 partitions via a host-prepared,
   row-shifted feat layout (featr), so each conv output needs only 3
   accumulating matmuls (kw taps).
 - Software pipelining: scores for kv-tile kt+1 are emitted before PV of
   kt so the PE never serializes behind the Activation engine. Conv for
   kv-block ln+1 is emitted inside the qc=0 attention stream as PE filler.

PSUM budget (8 banks x 2KB, bank-granular): sc 2x[128,1024] = 4,
U/D 4x[128,512] = 4 (a full bank per accumulator - an open accumulation
group must not share a bank with any other matmul group, or it loses
contributions on hardware). Conv/proj psum borrows the sc rotation.
"""

from contextlib import ExitStack

import numpy as np

import concourse.bass as bass
import concourse.mybir as mybir
import concourse.tile as tile

F32 = mybir.dt.float32
F32R = mybir.dt.float32r
BF16 = mybir.dt.bfloat16
AF = mybir.ActivationFunctionType
ALU = mybir.AluOpType

B = 2
C = 256
NH = 8
HD = 32
H = W = 64
L = H * W            # 4096 query / kv positions per batch
HF = WF = 128        # feat spatial
CF = 29              # feat channels actually used by the module
NCORE = 8
QCHUNK = L // 4      # 1024 queries per core
QN = 256             # attention q sub-chunk
NQC = QCHUNK // QN   # 4
KT = L // 128        # 32 kv tiles of 128
SCALE = float(1.0 / np.sqrt(HD))
FP = 87              # conv partitions: 3 kh taps x 29 channels


def build_kernel(nc: bass.Bass):
    # featr[kh*29+c, rr, cc] = feat[c, 2*rr + kh - 1, cc - 1] (0 out of bounds)
    featr = nc.declare_dram_parameter("featr", [128, 64, 130], BF16, isOutput=False)
    srcq = nc.declare_dram_parameter("srcq", [128, 2, QCHUNK], F32, isOutput=False)
    wqt = nc.declare_dram_parameter("wqt", [128, 2, C], F32, isOutput=False)
    wot = nc.declare_dram_parameter("wot", [128, 2, C], F32, isOutput=False)
    wkc = nc.declare_dram_parameter("wkc", [128, 3, C], BF16, isOutput=False)
    wvc = nc.declare_dram_parameter("wvc", [128, 3, C], BF16, isOutput=False)
    bq2 = nc.declare_dram_parameter("bq2", [128, 2], F32, isOutput=False)
    bk2 = nc.declare_dram_parameter("bk2", [128, 2], F32, isOutput=False)
    boe = nc.declare_dram_parameter("boe", [128, 2], F32, isOutput=False)
    onesd = nc.declare_dram_parameter("onesd", [128, 32], BF16, isOutput=False)
    outq = nc.declare_dram_parameter("outq", [C, QCHUNK], F32, isOutput=True)

    with ExitStack() as ctx:
        ctx.enter_context(
            nc.allow_low_precision("float32r tiles carry full fp32 bits")
        )
        tc = ctx.enter_context(tile.TileContext(nc))
        const = ctx.enter_context(tc.tile_pool(name="const", bufs=1))
        work = ctx.enter_context(tc.tile_pool(name="work", bufs=2))
        pwork = ctx.enter_context(tc.tile_pool(name="pwork", bufs=4))
        psc = ctx.enter_context(tc.tile_pool(name="psc", bufs=2, space="PSUM"))
        pacc = ctx.enter_context(tc.tile_pool(name="pacc", bufs=4, space="PSUM"))

        # ---- input loads (split across sync + scalar DMA queues) ----
        featr_sb = const.tile([128, 64, 130], BF16, tag="featr")
        nc.sync.dma_start(featr_sb[:, 0:32, :], featr[:, 0:32, :])
        nc.scalar.dma_start(featr_sb[:, 32:64, :], featr[:, 32:64, :])
        srcf_sb = const.tile([128, 2, QCHUNK], F32, tag="srcf")
        nc.sync.dma_start(srcf_sb[:], srcq[:])
        srcr_sb = const.tile([128, 2, QCHUNK], F32R, tag="srcr")
        nc.sync.dma_start(srcr_sb[:], srcq[:].bitcast(F32R))
        wqt_sb = const.tile([128, 2, C], F32R, tag="wqt")
        nc.scalar.dma_start(wqt_sb[:], wqt[:].bitcast(F32R))
        wkc_sb = const.tile([128, 3, C], BF16, tag="wkc")
        nc.scalar.dma_start(wkc_sb[:], wkc[:])
        wvc_sb = const.tile([128, 3, C], BF16, tag="wvc")
        nc.scalar.dma_start(wvc_sb[:], wvc[:])
        wot_sb = const.tile([128, 2, C], F32R, tag="wot")
        nc.scalar.dma_start(wot_sb[:], wot[:].bitcast(F32R))
        bq2_sb = const.tile([128, 2], F32, tag="bq2")
        nc.scalar.dma_start(bq2_sb[:], bq2[:])
        bk2_sb = const.tile([128, 2], F32, tag="bk2")
        nc.scalar.dma_start(bk2_sb[:], bk2[:])
        boe_sb = const.tile([128, 2], F32, tag="boe")
        nc.scalar.dma_start(boe_sb[:], boe[:])
        ones_sb = const.tile([128, 32], BF16, tag="ones")
        nc.scalar.dma_start(ones_sb[:], onesd[:])

        qT_sb = const.tile([128, 2, QCHUNK], BF16, tag="qT")
        kT_sb = const.tile([128, 2, L], BF16, tag="kT")
        v_sb = const.tile([128, KT, C], BF16, tag="v")

        # ---- Q^T = Wq @ src_chunk + bq   -> [C(part, 2 jo), QCHUNK] ----
        for jo in range(2):
            for qn in range(4):
                ps = psc.tile([128, 1024], F32, tag="sc", name=f"q{jo}{qn}")[:, 0:QN]
                for ki in range(2):
                    nc.tensor.matmul(
                        ps[:],
                        wqt_sb[:, ki, jo * 128 : (jo + 1) * 128],
                        srcr_sb[:, ki, qn * QN : (qn + 1) * QN],
                        start=(ki == 0),
                        stop=(ki == 1),
                    )
                nc.vector.tensor_scalar_add(
                    qT_sb[:, jo, qn * QN : (qn + 1) * QN], ps[:], bq2_sb[:, jo : jo + 1]
                )

        # ---- conv emitter: kv block ln covers oh in [8ln, 8ln+8) ----
        def emit_conv(ln):
            # K^T: out [C(2 jo x 128), 2 x 256 kv]
            for jo in range(2):
                for s in range(2):
                    ps = psc.tile([128, 1024], F32, tag="sc", name=f"k{ln}{jo}{s}")[:, 0:QN]
                    for kw in range(3):
                        nc.tensor.matmul(
                            ps[:],
                            wkc_sb[0:FP, kw, jo * 128 : (jo + 1) * 128],
                            featr_sb[
                                0:FP, 8 * ln + 4 * s : 8 * ln + 4 * s + 4,
                                kw : kw + 128 : 2,
                            ],
                            start=(kw == 0),
                            stop=(kw == 2),
                        )
                    nc.vector.tensor_scalar_add(
                        kT_sb[:, jo, 512 * ln + QN * s : 512 * ln + QN * (s + 1)],
                        ps[:],
                        bk2_sb[:, jo : jo + 1],
                    )
            # V: out [kv 128, C] for lt in the 4 kv-128 tiles of this block
            # (stationary operand must be 2D -> two 64-row halves, col-tiled)
            for lt in range(4 * ln, 4 * ln + 4):
                ps = psc.tile([128, 1024], F32, tag="sc", name=f"v{lt}")[:, 0:QN]
                for half in range(2):
                    for kw in range(3):
                        nc.tensor.matmul(
                            ps[64 * half : 64 * half + 64, :],
                            featr_sb[0:FP, 2 * lt + half, kw : kw + 128 : 2],
                            wvc_sb[0:FP, kw, :],
                            start=(kw == 0),
                            stop=(kw == 2),
                            tile_position=(0, 64 * half),
                            skip_group_check=True,
                        )
                nc.vector.tensor_copy(v_sb[:, lt, :], ps[:])

        emit_conv(0)

        # ---- attention: 4 q chunks x 32 kv tiles, software-pipelined ----
        def emit_scores_exp(qc, kt):
            p_tiles = []
            for t in range(2):
                sc = psc.tile([128, 1024], F32, tag="sc", name=f"s{qc}_{kt}_{t}")
                for g in (2 * t, 2 * t + 1):
                    for jo in range(2):
                        col = (2 * (g % 2) + jo) * QN
                        nc.tensor.matmul(
                            sc[:, col : col + QN],
                            kT_sb[32 * g : 32 * g + 32, jo, kt * 128 : (kt + 1) * 128],
                            qT_sb[32 * g : 32 * g + 32, jo, qc * QN : (qc + 1) * QN],
                            start=True,
                            stop=True,
                            tile_position=(32 * g, 0),
                            skip_group_check=True,
                        )
                p_sb = pwork.tile([128, 1024], BF16, tag="p", name=f"p{qc}_{kt}_{t}")
                nc.scalar.activation(p_sb[:], sc[:], AF.Exp, scale=SCALE)
                p_tiles.append(p_sb)
            return p_tiles

        def emit_pv(kt, p_tiles, u_t, d_t):
            for h in range(NH):
                g, jo = h % 4, h // 4
                psl = p_tiles[g // 2][:, (2 * (g % 2) + jo) * QN :][:, 0:QN]
                nc.tensor.matmul(
                    u_t[jo][32 * g : 32 * g + 32, 0:QN],
                    v_sb[:, kt, 32 * h : 32 * h + 32],
                    psl,
                    start=(kt == 0),
                    stop=(kt == KT - 1),
                    tile_position=(0, 32 * g),
                    skip_group_check=True,
                )
                nc.tensor.matmul(
                    d_t[jo][32 * g : 32 * g + 32, :],
                    ones_sb[:, 0:32],
                    psl,
                    start=(kt == 0),
                    stop=(kt == KT - 1),
                    tile_position=(0, 32 * g),
                    skip_group_check=True,
                )

        def emit_epilogue(qc, u_t, d_t):
            # normalize: o = U * (1/D) (D broadcast across each group's rows)
            rec_sb = work.tile([128, 2, QN], F32, tag="rec", name=f"rec{qc}")
            o_sb = work.tile([128, 2, QN], F32R, tag="o", name=f"o{qc}")
            for jo in range(2):
                nc.vector.reciprocal(rec_sb[:, jo, :], d_t[jo][:])
                nc.vector.tensor_tensor(
                    o_sb[:, jo, :], u_t[jo][:, 0:QN], rec_sb[:, jo, :], ALU.mult
                )
            # out projection + bias + * src, then store
            for jo in range(2):
                op = psc.tile([128, 1024], F32, tag="sc", name=f"op{qc}_{jo}")[:, 0:QN]
                for ki in range(2):
                    nc.tensor.matmul(
                        op[:],
                        wot_sb[:, ki, jo * 128 : (jo + 1) * 128],
                        o_sb[:, ki, :],
                        start=(ki == 0),
                        stop=(ki == 1),
                    )
                ot = work.tile([128, QN], F32, tag="ot", name=f"ot{qc}_{jo}")
                nc.vector.scalar_tensor_tensor(
                    ot[:],
                    op[:],
                    boe_sb[:, jo : jo + 1],
                    srcf_sb[:, jo, qc * QN : (qc + 1) * QN],
                    op0=ALU.add,
                    op1=ALU.mult,
                )
                nc.gpsimd.dma_start(
                    outq[jo * 128 : (jo + 1) * 128, qc * QN : (qc + 1) * QN], ot[:]
                )

        prev = None      # (kt, p_tiles, u_t, d_t) pending PV
        prev_epi = None  # (qc, u_t, d_t) pending epilogue
        for qc in range(NQC):
            u_t = [
                pacc.tile([128, 512], F32, tag="u", name=f"u{qc}_{jo}")
                for jo in range(2)
            ]
            d_t = [
                pacc.tile([128, 512], F32, tag="u", name=f"d{qc}_{jo}")[:, 0:QN]
                for jo in range(2)
            ]
            for kt in range(KT):
                p_tiles = emit_scores_exp(qc, kt)
                if prev is not None:
                    emit_pv(*prev)
                if prev_epi is not None:
                    emit_epilogue(*prev_epi)
                    prev_epi = None
                prev = (kt, p_tiles, u_t, d_t)
                # interleave conv for the next kv block into the qc=0 stream
                if qc == 0 and kt % 4 == 1 and kt // 4 < 7:
                    emit_conv(kt // 4 + 1)
            prev_epi = (qc, u_t, d_t)
        emit_pv(*prev)
        emit_epilogue(*prev_epi)

    return nc


_CACHE: dict = {}


def _split_matmul_waits(nc: bass.Bass):
    """walrus's fp32r self-loading matmul (S3 LW struct) accepts only one
    sync-wait command; peel extra waits onto PE EventSemaphore ops inserted
    immediately before the matmul (same sync point, so no deadlock risk)."""
    import bass_rust

    n_new = 0
    for fn in nc.m.functions:
        for block in fn.blocks:
            insts = list(block.instructions)
            out = []
            changed = False
            skip = (
                mybir.InstEventSemaphore,
                mybir.InstAllEngineBarrier,
                mybir.InstHalt,
            )
            for inst in insts:
                if not isinstance(inst, skip) and inst.sync_info is not None:
                    si = inst.sync_info
                    waits = list(si.on_wait)
                    if len(waits) > 1:
                        for w in waits[:-1]:
                            ev = mybir.InstEventSemaphore(
                                name=f"WSPLIT-{n_new}", ins=[], outs=[]
                            )
                            ev.engine = inst.engine
                            ev.sync_info = bass_rust.SyncInfo(
                                on_wait=[w], on_update=[]
                            )
                            out.append(ev)
                            n_new += 1
                        inst.sync_info = bass_rust.SyncInfo(
                            on_wait=[waits[-1]], on_update=list(si.on_update)
                        )
                        changed = True
                out.append(inst)
            if changed:
                block.instructions = out
    return n_new


def get_nc() -> bass.Bass:
    if "nc" not in _CACHE:
        nc = bass.Bass()
        build_kernel(nc)
        _split_matmul_waits(nc)
        nc.finalize()
        _CACHE["nc"] = nc
    return _CACHE["nc"]


def make_core_inputs(feat, src, Wq, bq, Wk, bk, Wv, bv, Wo, bo):
    """Host-side sharding / layout prep. Returns list of 8 input dicts."""
    import ml_dtypes

    f32 = np.float32
    bf16 = ml_dtypes.bfloat16
    feat = np.asarray(feat, f32)
    src = np.asarray(src, f32)
    Wq, Wk, Wv, Wo = (np.asarray(x, f32) for x in (Wq, Wk, Wv, Wo))
    bq, bk, bv, bo = (np.asarray(x, f32) for x in (bq, bk, bv, bo))

    wqt = np.ascontiguousarray(Wq.T.reshape(2, 128, C).transpose(1, 0, 2))
    wot = np.ascontiguousarray(Wo.T.reshape(2, 128, C).transpose(1, 0, 2))

    # conv-tap layouts: wkc[kh*29+c, kw, cout] = Wk[cout, 9c+3kh+kw] (0 pad)
    wkc = np.zeros((128, 3, C), f32)
    wvc = np.zeros((128, 3, C), f32)
    for kh in range(3):
        for kw in range(3):
            for c in range(CF):
                j = 9 * c + 3 * kh + kw
                if j < C:
                    wkc[kh * CF + c, kw, :] = Wk[:, j]
                    wvc[kh * CF + c, kw, :] = Wv[:, j]
    wkc = wkc.astype(bf16)
    wvc = wvc.astype(bf16)
    onesd = np.ones((128, 32), bf16)

    bq2 = np.ascontiguousarray(bq.reshape(2, 128).T)
    bk2 = np.ascontiguousarray(bk.reshape(2, 128).T)
    boev = Wo @ bv + bo
    boe = np.ascontiguousarray(boev.reshape(2, 128).T)

    shared = dict(
        wqt=wqt, wot=wot, wkc=wkc, wvc=wvc, bq2=bq2, bk2=bk2, boe=boe, onesd=onesd
    )

    # featr[kh*29+c, rr, cc] = feat[b, c, 2rr+kh-1, cc-1], 0 out of bounds
    featr_b = []
    rr = np.arange(64)
    for b in range(B):
        fr = np.zeros((128, 64, 130), f32)
        for kh in range(3):
            rows = 2 * rr + kh - 1
            valid = (rows >= 0) & (rows < HF)
            blk = np.zeros((CF, 64, 130), f32)
            blk[:, valid, 1:129] = feat[b, :CF][:, rows[valid], :]
            fr[kh * CF : kh * CF + CF] = blk
        featr_b.append(fr.astype(bf16))

    in_maps = []
    for core in range(NCORE):
        b, qi = divmod(core, 4)
        m = dict(shared)
        m["featr"] = featr_b[b]
        sl = src[b].reshape(C, L)[:, qi * QCHUNK : (qi + 1) * QCHUNK]
        m["srcq"] = np.ascontiguousarray(
            sl.reshape(2, 128, QCHUNK).transpose(1, 0, 2)
        )
        in_maps.append(m)
    return in_maps


def _ensure_ntff_hook():
    """Provide antenv.axon_hooks if the image lacks it (needed for trace=True)."""
    import contextlib
    import ctypes
    import os
    import sys
    import types

    try:
        import antenv.axon_hooks  # noqa: F401

        return
    except ImportError:
        pass

    mod = types.ModuleType("antenv.axon_hooks")
    box = [None]
    mod.set_axon_ntff_profile_hook = lambda h: box.__setitem__(0, h)
    mod.get_axon_ntff_profile_hook = lambda: box[0]
    sys.modules["antenv.axon_hooks"] = mod
    import antenv

    antenv.axon_hooks = mod

    so_path = os.environ.get("PJRT_LIBRARY_PATH", "/opt/axon/libaxon_pjrt.so")
    try:
        lib = ctypes.CDLL(so_path)
    except OSError:
        return
    if not hasattr(lib, "axon_start_nrt_profile"):
        return
    lib.axon_start_nrt_profile.argtypes = [
        ctypes.POINTER(ctypes.c_int64),
        ctypes.c_size_t,
    ]
    lib.axon_start_nrt_profile.restype = ctypes.c_int64
    lib.axon_stop_nrt_profile.argtypes = [ctypes.c_char_p]
    lib.axon_stop_nrt_profile.restype = ctypes.c_int64

    @contextlib.contextmanager
    def _hook(output_dir, device_ids):
        import jax

        jax.devices()
        if device_ids:
            ids = (ctypes.c_int64 * len(device_ids))(*device_ids)
            rc = lib.axon_start_nrt_profile(ids, len(device_ids))
        else:
            rc = lib.axon_start_nrt_profile(None, 0)
        if rc != 0:
            raise RuntimeError(f"axon_start_nrt_profile rc={rc}")
        try:
            yield
        finally:
            n = lib.axon_stop_nrt_profile(str(output_dir).encode())
            print(f"profile: {n} file(s) written to {output_dir}", file=sys.stderr)

    box[0] = _hook


def run(inputs: dict, trace: bool = False, trace_cores=None):
    _ensure_ntff_hook()
    from concourse.bass_utils import run_bass_kernel_spmd

    nc = get_nc()
    in_maps = make_core_inputs(**inputs)
    res = run_bass_kernel_spmd(
        nc,
        in_maps,
        list(range(NCORE)),
        trace=trace,
        trace_cores=trace_cores,
    )
    out = np.empty((B, C, L), np.float32)
    for core in range(NCORE):
        b, qi = divmod(core, 4)
        out[b, :, qi * QCHUNK : (qi + 1) * QCHUNK] = res.results[core]["outq"]
    return out.reshape(B, C, H, W), res


def kernel(feat, src, Wq, bq, Wk, bk, Wv, bv, Wo, bo):
    out, _ = run(
        dict(feat=feat, src=src, Wq=Wq, bq=bq, Wk=Wk, bk=bk, Wv=Wv, bv=bv, Wo=Wo, bo=bo)
    )
    return out


# revision 16
# speedup vs baseline: 2.1428x; 1.0009x over previous
"""Trainium2 Bass kernel for the CSSAM sparse-attention module (v2).

Math (per batch b):
  q_in  = src[b] viewed as [C, L] (L = 64*64 = 4096)               (queries)
  kv[j, l] = featpad[b, j//9, kh + 2*oh - 1, kw + 2*ow - 1]
             where (kh, kw) = divmod(j % 9, 3), l = oh*64 + ow     (keys/vals)
      -> only feat channels 0..28 are ever used
  Q^T = Wq @ q_in + bq ; K^T = Wk @ kv + bk ; V = kv^T Wv^T
  per head h (8 heads, d = 32): softmax((Qh^T)^T Kh / sqrt(d)) Vh
  out[b] = (Wo @ O^T + (Wo bv + bo)) * src[b]

Sharding: 8 cores = 2 batches x 4 query-chunks of 1024; K/V work is
replicated across the 4 cores of a batch.

v2 design notes (HW-model driven):
 - The kernel is Activation-engine bound: exp over 8 heads x 4096 kv x
   1024 q per core = 262144 rows/partition ~ 218 us busy minimum. All
   other engines are paced to hide under it.
 - Scores matmuls use 4-way row-group concurrency (tile_position=(32g,0));
   PV + denominator use 4-way col-group concurrency (tile_position=(0,32g)).
 - Denominator rows are broadcast to all 32 partitions of a group by a
   [128, 32] ones lhsT (M=32), so normalization is a plain elementwise
   multiply by 1/D. D accumulates in its own PSUM bank - interleaving the
   D accumulation group into the U bank corrupts U on hardware.
 - K/V convs pack (channel, kh) into 87 partitions via a host-prepared,
   row-shifted feat layout (featr), so each conv output needs only 3
   accumulating matmuls (kw taps).
 - Software pipelining: scores for kv-tile kt+1 are emitted before PV of
   kt so the PE never serializes behind the Activation engine. Conv for
   kv-block ln+1 is emitted inside the qc=0 attention stream as PE filler.

PSUM budget (8 banks x 2KB, bank-granular): sc 2x[128,1024] = 4,
U/D 4x[128,512] = 4 (a full bank per accumulator - an open accumulation
group must not share a bank with any other matmul group, or it loses
contributions on hardware). Conv/proj psum borrows the sc rotation.
"""

from contextlib import ExitStack

import numpy as np

import concourse.bass as bass
import concourse.mybir as mybir
import concourse.tile as tile

F32 = mybir.dt.float32
F32R = mybir.dt.float32r
BF16 = mybir.dt.bfloat16
AF = mybir.ActivationFunctionType
ALU = mybir.AluOpType

B = 2
C = 256
NH = 8
HD = 32
H = W = 64
L = H * W            # 4096 query / kv positions per batch
HF = WF = 128        # feat spatial
CF = 29              # feat channels actually used by the module
NCORE = 8
QCHUNK = L // 4      # 1024 queries per core
QN = 256             # attention q sub-chunk
NQC = QCHUNK // QN   # 4
KT = L // 128        # 32 kv tiles of 128
SCALE = float(1.0 / np.sqrt(HD))
FP = 87              # conv partitions: 3 kh taps x 29 channels


def build_kernel(nc: bass.Bass):
    # featr[kh*29+c, rr, cc] = feat[c, 2*rr + kh - 1, cc - 1] (0 out of bounds)
    featr = nc.declare_dram_parameter("featr", [128, 64, 130], BF16, isOutput=False)
    srcq = nc.declare_dram_parameter("srcq", [128, 2, QCHUNK], F32, isOutput=False)
    wqt = nc.declare_dram_parameter("wqt", [128, 2, C], F32, isOutput=False)
    wot = nc.declare_dram_parameter("wot", [128, 2, C], F32, isOutput=False)
    wkc = nc.declare_dram_parameter("wkc", [128, 3, C], BF16, isOutput=False)
    wvc = nc.declare_dram_parameter("wvc", [128, 3, C], BF16, isOutput=False)
    bq2 = nc.declare_dram_parameter("bq2", [128, 2], F32, isOutput=False)
    bk2 = nc.declare_dram_parameter("bk2", [128, 2], F32, isOutput=False)
    boe = nc.declare_dram_parameter("boe", [128, 2], F32, isOutput=False)
    onesd = nc.declare_dram_parameter("onesd", [128, 32], BF16, isOutput=False)
    outq = nc.declare_dram_parameter("outq", [C, QCHUNK], F32, isOutput=True)

    with ExitStack() as ctx:
        ctx.enter_context(
            nc.allow_low_precision("float32r tiles carry full fp32 bits")
        )
        tc = ctx.enter_context(tile.TileContext(nc))
        const = ctx.enter_context(tc.tile_pool(name="const", bufs=1))
        work = ctx.enter_context(tc.tile_pool(name="work", bufs=2))
        pwork = ctx.enter_context(tc.tile_pool(name="pwork", bufs=4))
        psc = ctx.enter_context(tc.tile_pool(name="psc", bufs=2, space="PSUM"))
        pacc = ctx.enter_context(tc.tile_pool(name="pacc", bufs=4, space="PSUM"))

        # ---- exp table preload (hide the 1.3us ACT_TABLE_LOAD under DMAs) ----
        dummy = work.tile([128, 1], F32, tag="dummy", name="dummy")
        nc.gpsimd.memset(dummy[:], 0.0)
        nc.scalar.activation(dummy[:], dummy[:], AF.Exp)

        # ---- input loads, critical path first, split across DMA queues ----
        featr_sb = const.tile([128, 64, 130], BF16, tag="featr")
        wkc_sb = const.tile([128, 3, C], BF16, tag="wkc")
        wvc_sb = const.tile([128, 3, C], BF16, tag="wvc")
        wqt_sb = const.tile([128, 2, C], F32R, tag="wqt")
        wot_sb = const.tile([128, 2, C], F32R, tag="wot")
        srcf_sb = const.tile([128, 2, QCHUNK], F32, tag="srcf")
        srcr_sb = const.tile([128, 2, QCHUNK], F32R, tag="srcr")
        bq2_sb = const.tile([128, 2], F32, tag="bq2")
        bk2_sb = const.tile([128, 2], F32, tag="bk2")
        boe_sb = const.tile([128, 2], F32, tag="boe")
        ones_sb = const.tile([128, 32], BF16, tag="ones")
        # sync queue: feat half 1, then q-projection inputs
        nc.sync.dma_start(featr_sb[:, 0:32, :], featr[:, 0:32, :])
        nc.sync.dma_start(srcr_sb[:, :, 0:QN], srcq[:, :, 0:QN].bitcast(F32R))
        nc.sync.dma_start(srcr_sb[:, :, QN:], srcq[:, :, QN:].bitcast(F32R))
        nc.sync.dma_start(srcf_sb[:], srcq[:])
        # scalar queue: conv weights, q weights, feat half 2, the rest
        nc.scalar.dma_start(wkc_sb[:], wkc[:])
        nc.scalar.dma_start(wvc_sb[:], wvc[:])
        nc.scalar.dma_start(wqt_sb[:], wqt[:].bitcast(F32R))
        nc.scalar.dma_start(bq2_sb[:], bq2[:])
        nc.scalar.dma_start(bk2_sb[:], bk2[:])
        nc.scalar.dma_start(ones_sb[:], onesd[:])
        nc.scalar.dma_start(featr_sb[:, 32:64, :], featr[:, 32:64, :])
        nc.scalar.dma_start(wot_sb[:], wot[:].bitcast(F32R))
        nc.scalar.dma_start(boe_sb[:], boe[:])

        qT_sb = const.tile([128, 2, QCHUNK], BF16, tag="qT")
        kT_sb = const.tile([128, 2, L], BF16, tag="kT")
        v_sb = const.tile([128, KT, C], BF16, tag="v")

        # ---- Q^T = Wq @ src_chunk + bq   -> [C(part, 2 jo), QCHUNK] ----
        def emit_qproj(qn):
            for jo in range(2):
                ps = psc.tile([128, 1024], F32, tag="sc", name=f"q{jo}{qn}")[:, 0:QN]
                for ki in range(2):
                    nc.tensor.matmul(
                        ps[:],
                        wqt_sb[:, ki, jo * 128 : (jo + 1) * 128],
                        srcr_sb[:, ki, qn * QN : (qn + 1) * QN],
                        start=(ki == 0),
                        stop=(ki == 1),
                    )
                nc.vector.tensor_scalar_add(
                    qT_sb[:, jo, qn * QN : (qn + 1) * QN], ps[:], bq2_sb[:, jo : jo + 1]
                )

        # ---- conv emitters: kv block ln covers oh in [8ln, 8ln+8).
        # Each block = 4 K-quarters (jo x s) + 4 V kv-128 tiles; one
        # (K-quarter, V-tile) pair per attention kt keeps the PE filler
        # fine-grained and the psc rotation parity even. ----
        def emit_conv_k(ln, o):
            jo, s = divmod(o, 2)
            ps = psc.tile([128, 1024], F32, tag="sc", name=f"k{ln}{jo}{s}")[:, 0:QN]
            for kw in range(3):
                nc.tensor.matmul(
                    ps[:],
                    wkc_sb[0:FP, kw, jo * 128 : (jo + 1) * 128],
                    featr_sb[
                        0:FP, 8 * ln + 4 * s : 8 * ln + 4 * s + 4,
                        kw : kw + 128 : 2,
                    ],
                    start=(kw == 0),
                    stop=(kw == 2),
                )
            nc.vector.tensor_scalar_add(
                kT_sb[:, jo, 512 * ln + QN * s : 512 * ln + QN * (s + 1)],
                ps[:],
                bk2_sb[:, jo : jo + 1],
            )

        def emit_conv_v(lt):
            # stationary operand must be 2D -> two 64-row halves, col-tiled
            ps = psc.tile([128, 1024], F32, tag="sc", name=f"v{lt}")[:, 0:QN]
            for half in range(2):
                for kw in range(3):
                    nc.tensor.matmul(
                        ps[64 * half : 64 * half + 64, :],
                        featr_sb[0:FP, 2 * lt + half, kw : kw + 128 : 2],
                        wvc_sb[0:FP, kw, :],
                        start=(kw == 0),
                        stop=(kw == 2),
                        tile_position=(0, 64 * half),
                        skip_group_check=True,
                    )
            nc.vector.tensor_copy(v_sb[:, lt, :], ps[:])

        def emit_conv_block(ln):
            for o in range(4):
                emit_conv_k(ln, o)
            for lt in range(4 * ln, 4 * ln + 4):
                emit_conv_v(lt)

        emit_conv_block(0)
        emit_qproj(0)

        # ---- attention: 4 q chunks x 32 kv tiles, software-pipelined ----
        def emit_scores_exp(qc, kt):
            p_tiles = []
            for t in range(2):
                sc = psc.tile([128, 1024], F32, tag="sc", name=f"s{qc}_{kt}_{t}")
                for g in (2 * t, 2 * t + 1):
                    for jo in range(2):
                        col = (2 * (g % 2) + jo) * QN
                        nc.tensor.matmul(
                            sc[:, col : col + QN],
                            kT_sb[32 * g : 32 * g + 32, jo, kt * 128 : (kt + 1) * 128],
                            qT_sb[32 * g : 32 * g + 32, jo, qc * QN : (qc + 1) * QN],
                            start=True,
                            stop=True,
                            tile_position=(32 * g, 0),
                            skip_group_check=True,
                        )
                p_sb = pwork.tile([128, 1024], BF16, tag="p", name=f"p{qc}_{kt}_{t}")
                nc.scalar.activation(p_sb[:], sc[:], AF.Exp, scale=SCALE)
                p_tiles.append(p_sb)
            return p_tiles

        def emit_pv(kt, p_tiles, u_t, d_t):
            for h in range(NH):
                g, jo = h % 4, h // 4
                psl = p_tiles[g // 2][:, (2 * (g % 2) + jo) * QN :][:, 0:QN]
                nc.tensor.matmul(
                    u_t[jo][32 * g : 32 * g + 32, 0:QN],
                    v_sb[:, kt, 32 * h : 32 * h + 32],
                    psl,
                    start=(kt == 0),
                    stop=(kt == KT - 1),
                    tile_position=(0, 32 * g),
                    skip_group_check=True,
                )
                nc.tensor.matmul(
                    d_t[jo][32 * g : 32 * g + 32, :],
                    ones_sb[:, 0:32],
                    psl,
                    start=(kt == 0),
                    stop=(kt == KT - 1),
                    tile_position=(0, 32 * g),
                    skip_group_check=True,
                )

        def emit_epilogue(qc, u_t, d_t):
            # normalize: o = U * (1/D) (D broadcast across each group's rows)
            rec_sb = work.tile([128, 2, QN], F32, tag="rec", name=f"rec{qc}")
            o_sb = work.tile([128, 2, QN], F32R, tag="o", name=f"o{qc}")
            for jo in range(2):
                nc.vector.reciprocal(rec_sb[:, jo, :], d_t[jo][:])
                nc.vector.tensor_tensor(
                    o_sb[:, jo, :], u_t[jo][:, 0:QN], rec_sb[:, jo, :], ALU.mult
                )
            # out projection + bias + * src, then store
            for jo in range(2):
                op = psc.tile([128, 1024], F32, tag="sc", name=f"op{qc}_{jo}")[:, 0:QN]
                for ki in range(2):
                    nc.tensor.matmul(
                        op[:],
                        wot_sb[:, ki, jo * 128 : (jo + 1) * 128],
                        o_sb[:, ki, :],
                        start=(ki == 0),
                        stop=(ki == 1),
                    )
                ot = work.tile([128, QN], F32, tag="ot", name=f"ot{qc}_{jo}")
                nc.vector.scalar_tensor_tensor(
                    ot[:],
                    op[:],
                    boe_sb[:, jo : jo + 1],
                    srcf_sb[:, jo, qc * QN : (qc + 1) * QN],
                    op0=ALU.add,
                    op1=ALU.mult,
                )
                nc.gpsimd.dma_start(
                    outq[jo * 128 : (jo + 1) * 128, qc * QN : (qc + 1) * QN], ot[:]
                )

        prev = None      # (kt, p_tiles, u_t, d_t) pending PV
        prev_epi = None  # (qc, u_t, d_t) pending epilogue
        for qc in range(NQC):
            u_t = [
                pacc.tile([128, 512], F32, tag="u", name=f"u{qc}_{jo}")
                for jo in range(2)
            ]
            d_t = [
                pacc.tile([128, 512], F32, tag="u", name=f"d{qc}_{jo}")[:, 0:QN]
                for jo in range(2)
            ]
            for kt in range(KT):
                # interleave conv for kv block kt//4+1 into the qc=0 stream,
                # one (K-quarter, V-tile) pair per kt; emitted before scores
                # so the psc rotation parity keeps scores 2 tiles deep
                if qc == 0 and kt < 28:
                    ln, o = kt // 4 + 1, kt % 4
                    emit_conv_k(ln, o)
                    emit_conv_v(4 * ln + o)
                p_tiles = emit_scores_exp(qc, kt)
                if prev is not None:
                    emit_pv(*prev)
                if prev_epi is not None:
                    emit_epilogue(*prev_epi)
                    prev_epi = None
                prev = (kt, p_tiles, u_t, d_t)
                if kt == 16 and qc < NQC - 1:
                    emit_qproj(qc + 1)
            prev_epi = (qc, u_t, d_t)
        emit_pv(*prev)
        emit_epilogue(*prev_epi)

    return nc


_CACHE: dict = {}


def _split_matmul_waits(nc: bass.Bass):
    """walrus's fp32r self-loading matmul (S3 LW struct) accepts only one
    sync-wait command; peel extra waits onto PE EventSemaphore ops inserted
    immediately before the matmul (same sync point, so no deadlock risk)."""
    import bass_rust

    n_new = 0
    for fn in nc.m.functions:
        for block in fn.blocks:
            insts = list(block.instructions)
            out = []
            changed = False
            skip = (
                mybir.InstEventSemaphore,
                mybir.InstAllEngineBarrier,
                mybir.InstHalt,
            )
            for inst in insts:
                if not isinstance(inst, skip) and inst.sync_info is not None:
                    si = inst.sync_info
                    waits = list(si.on_wait)
                    if len(waits) > 1:
                        for w in waits[:-1]:
                            ev = mybir.InstEventSemaphore(
                                name=f"WSPLIT-{n_new}", ins=[], outs=[]
                            )
                            ev.engine = inst.engine
                            ev.sync_info = bass_rust.SyncInfo(
                                on_wait=[w], on_update=[]
                            )
                            out.append(ev)
                            n_new += 1
                        inst.sync_info = bass_rust.SyncInfo(
                            on_wait=[waits[-1]], on_update=list(si.on_update)
                        )
                        changed = True
                out.append(inst)
            if changed:
                block.instructions = out
    return n_new


def get_nc() -> bass.Bass:
    if "nc" not in _CACHE:
        nc = bass.Bass()
        build_kernel(nc)
        _split_matmul_waits(nc)
        nc.finalize()
        _CACHE["nc"] = nc
    return _CACHE["nc"]


def make_core_inputs(feat, src, Wq, bq, Wk, bk, Wv, bv, Wo, bo):
    """Host-side sharding / layout prep. Returns list of 8 input dicts."""
    import ml_dtypes

    f32 = np.float32
    bf16 = ml_dtypes.bfloat16
    feat = np.asarray(feat, f32)
    src = np.asarray(src, f32)
    Wq, Wk, Wv, Wo = (np.asarray(x, f32) for x in (Wq, Wk, Wv, Wo))
    bq, bk, bv, bo = (np.asarray(x, f32) for x in (bq, bk, bv, bo))

    wqt = np.ascontiguousarray(Wq.T.reshape(2, 128, C).transpose(1, 0, 2))
    wot = np.ascontiguousarray(Wo.T.reshape(2, 128, C).transpose(1, 0, 2))

    # conv-tap layouts: wkc[kh*29+c, kw, cout] = Wk[cout, 9c+3kh+kw] (0 pad)
    wkc = np.zeros((128, 3, C), f32)
    wvc = np.zeros((128, 3, C), f32)
    for kh in range(3):
        for kw in range(3):
            for c in range(CF):
                j = 9 * c + 3 * kh + kw
                if j < C:
                    wkc[kh * CF + c, kw, :] = Wk[:, j]
                    wvc[kh * CF + c, kw, :] = Wv[:, j]
    wkc = wkc.astype(bf16)
    wvc = wvc.astype(bf16)
    onesd = np.ones((128, 32), bf16)

    bq2 = np.ascontiguousarray(bq.reshape(2, 128).T)
    bk2 = np.ascontiguousarray(bk.reshape(2, 128).T)
    boev = Wo @ bv + bo
    boe = np.ascontiguousarray(boev.reshape(2, 128).T)

    shared = dict(
        wqt=wqt, wot=wot, wkc=wkc, wvc=wvc, bq2=bq2, bk2=bk2, boe=boe, onesd=onesd
    )

    # featr[kh*29+c, rr, cc] = feat[b, c, 2rr+kh-1, cc-1], 0 out of bounds
    featr_b = []
    rr = np.arange(64)
    for b in range(B):
        fr = np.zeros((128, 64, 130), f32)
        for kh in range(3):
            rows = 2 * rr + kh - 1
            valid = (rows >= 0) & (rows < HF)
            blk = np.zeros((CF, 64, 130), f32)
            blk[:, valid, 1:129] = feat[b, :CF][:, rows[valid], :]
            fr[kh * CF : kh * CF + CF] = blk
        featr_b.append(fr.astype(bf16))

    in_maps = []
    for core in range(NCORE):
        b, qi = divmod(core, 4)
        m = dict(shared)
        m["featr"] = featr_b[b]
        sl = src[b].reshape(C, L)[:, qi * QCHUNK : (qi + 1) * QCHUNK]
        m["srcq"] = np.ascontiguousarray(
            sl.reshape(2, 128, QCHUNK).transpose(1, 0, 2)
        )
        in_maps.append(m)
    return in_maps


def _ensure_ntff_hook():
    """Provide antenv.axon_hooks if the image lacks it (needed for trace=True)."""
    import contextlib
    import ctypes
    import os
    import sys
    import types

    try:
        import antenv.axon_hooks  # noqa: F401

        return
    except ImportError:
        pass

    mod = types.ModuleType("antenv.axon_hooks")
    box = [None]
    mod.set_axon_ntff_profile_hook = lambda h: box.__setitem__(0, h)
    mod.get_axon_ntff_profile_hook = lambda: box[0]
    sys.modules["antenv.axon_hooks"] = mod
    import antenv

    antenv.axon_hooks = mod

    so_path = os.environ.get("PJRT_LIBRARY_PATH", "/opt/axon/libaxon_pjrt.so")
    try:
        lib = ctypes.CDLL(so_path)
    except OSError:
        return
    if not hasattr(lib, "axon_start_nrt_profile"):
        return
    lib.axon_start_nrt_profile.argtypes = [
        ctypes.POINTER(ctypes.c_int64),
        ctypes.c_size_t,
    ]
    lib.axon_start_nrt_profile.restype = ctypes.c_int64
    lib.axon_stop_nrt_profile.argtypes = [ctypes.c_char_p]
    lib.axon_stop_nrt_profile.restype = ctypes.c_int64

    @contextlib.contextmanager
    def _hook(output_dir, device_ids):
        import jax

        jax.devices()
        if device_ids:
            ids = (ctypes.c_int64 * len(device_ids))(*device_ids)
            rc = lib.axon_start_nrt_profile(ids, len(device_ids))
        else:
            rc = lib.axon_start_nrt_profile(None, 0)
        if rc != 0:
            raise RuntimeError(f"axon_start_nrt_profile rc={rc}")
        try:
            yield
        finally:
            n = lib.axon_stop_nrt_profile(str(output_dir).encode())
            print(f"profile: {n} file(s) written to {output_dir}", file=sys.stderr)

    box[0] = _hook


def run(inputs: dict, trace: bool = False, trace_cores=None):
    _ensure_ntff_hook()
    from concourse.bass_utils import run_bass_kernel_spmd

    nc = get_nc()
    in_maps = make_core_inputs(**inputs)
    res = run_bass_kernel_spmd(
        nc,
        in_maps,
        list(range(NCORE)),
        trace=trace,
        trace_cores=trace_cores,
    )
    out = np.empty((B, C, L), np.float32)
    for core in range(NCORE):
        b, qi = divmod(core, 4)
        out[b, :, qi * QCHUNK : (qi + 1) * QCHUNK] = res.results[core]["outq"]
    return out.reshape(B, C, H, W), res


def kernel(feat, src, Wq, bq, Wk, bk, Wv, bv, Wo, bo):
    out, _ = run(
        dict(feat=feat, src=src, Wq=Wq, bq=bq, Wk=Wk, bk=bk, Wv=Wv, bv=bv, Wo=Wo, bo=bo)
    )
    return out


# revision 17
# speedup vs baseline: 2.2461x; 1.0482x over previous
"""Trainium2 Bass kernel for the CSSAM sparse-attention module (v2).

Math (per batch b):
  q_in  = src[b] viewed as [C, L] (L = 64*64 = 4096)               (queries)
  kv[j, l] = featpad[b, j//9, kh + 2*oh - 1, kw + 2*ow - 1]
             where (kh, kw) = divmod(j % 9, 3), l = oh*64 + ow     (keys/vals)
      -> only feat channels 0..28 are ever used
  Q^T = Wq @ q_in + bq ; K^T = Wk @ kv + bk ; V = kv^T Wv^T
  per head h (8 heads, d = 32): softmax((Qh^T)^T Kh / sqrt(d)) Vh
  out[b] = (Wo @ O^T + (Wo bv + bo)) * src[b]

Sharding: 8 cores = 2 batches x 4 query-chunks of 1024; K/V work is
replicated across the 4 cores of a batch.

v2 design notes (HW-model driven):
 - The kernel is Activation-engine bound: exp over 8 heads x 4096 kv x
   1024 q per core = 262144 rows/partition ~ 218 us busy minimum. All
   other engines are paced to hide under it.
 - Scores matmuls use 4-way row-group concurrency (tile_position=(32g,0));
   PV + denominator use 4-way col-group concurrency (tile_position=(0,32g)).
 - Denominator rows are broadcast to all 32 partitions of a group by a
   [128, 32] ones lhsT (M=32), so normalization is a plain elementwise
   multiply by 1/D. D accumulates in its own PSUM bank - interleaving the
   D accumulation group into the U bank corrupts U on hardware.
 - K/V convs pack (channel, kh) into 87 partitions via a host-prepared,
   per-tap pre-shifted im2col layout (featv), so each conv output needs
   only 3 accumulating matmuls (kw taps) and both convs are plain 2D.
 - Software pipelining: scores for kv-tile kt+1 are emitted before PV of
   kt so the PE never serializes behind the Activation engine.

PSUM budget (8 banks x 2KB, bank-granular): sc 2x[128,1024] = 4,
U/D 4x[128,512] = 4 (a full bank per accumulator - an open accumulation
group must not share a bank with any other matmul group, or it loses
contributions on hardware). Conv/proj psum borrows the sc rotation.
"""

from contextlib import ExitStack

import numpy as np

import concourse.bass as bass
import concourse.mybir as mybir
import concourse.tile as tile

F32 = mybir.dt.float32
F32R = mybir.dt.float32r
BF16 = mybir.dt.bfloat16
AF = mybir.ActivationFunctionType
ALU = mybir.AluOpType

B = 2
C = 256
NH = 8
HD = 32
H = W = 64
L = H * W            # 4096 query / kv positions per batch
HF = WF = 128        # feat spatial
CF = 29              # feat channels actually used by the module
NCORE = 8
QCHUNK = L // 4      # 1024 queries per core
QN = 256             # attention q sub-chunk
NQC = QCHUNK // QN   # 4
KT = L // 128        # 32 kv tiles of 128
SCALE = float(1.0 / np.sqrt(HD))
FP = 87              # conv partitions: 3 kh taps x 29 channels


def build_kernel(nc: bass.Bass):
    # featv[kh*29+c, kw, oh*64+ow] = feat[c, 2*oh + kh - 1, 2*ow + kw - 1]
    # (0 out of bounds): per-tap pre-shifted im2col planes, host-prepared
    featv = nc.declare_dram_parameter("featv", [128, 3, L], BF16, isOutput=False)
    srcq = nc.declare_dram_parameter("srcq", [128, 2, QCHUNK], F32, isOutput=False)
    wqt = nc.declare_dram_parameter("wqt", [128, 2, C], F32, isOutput=False)
    wot = nc.declare_dram_parameter("wot", [128, 2, C], F32, isOutput=False)
    wkc = nc.declare_dram_parameter("wkc", [128, 3, C], BF16, isOutput=False)
    wvc = nc.declare_dram_parameter("wvc", [128, 3, C], BF16, isOutput=False)
    bq2 = nc.declare_dram_parameter("bq2", [128, 2], F32, isOutput=False)
    bk2 = nc.declare_dram_parameter("bk2", [128, 2], F32, isOutput=False)
    boe = nc.declare_dram_parameter("boe", [128, 2], F32, isOutput=False)
    onesd = nc.declare_dram_parameter("onesd", [128, 32], BF16, isOutput=False)
    outq = nc.declare_dram_parameter("outq", [C, QCHUNK], F32, isOutput=True)

    with ExitStack() as ctx:
        ctx.enter_context(
            nc.allow_low_precision("float32r tiles carry full fp32 bits")
        )
        tc = ctx.enter_context(tile.TileContext(nc))
        const = ctx.enter_context(tc.tile_pool(name="const", bufs=1))
        work = ctx.enter_context(tc.tile_pool(name="work", bufs=2))
        pwork = ctx.enter_context(tc.tile_pool(name="pwork", bufs=4))
        psc = ctx.enter_context(tc.tile_pool(name="psc", bufs=2, space="PSUM"))
        pacc = ctx.enter_context(tc.tile_pool(name="pacc", bufs=4, space="PSUM"))

        # ---- exp table preload (hide the 1.3us ACT_TABLE_LOAD under DMAs) ----
        dummy = work.tile([128, 1], F32, tag="dummy", name="dummy")
        nc.gpsimd.memset(dummy[:], 0.0)
        nc.scalar.activation(dummy[:], dummy[:], AF.Exp)

        # ---- input loads, critical path first, split across DMA queues ----
        featv_sb = const.tile([128, 3, L], BF16, tag="featv")
        wkc_sb = const.tile([128, 3, C], BF16, tag="wkc")
        wvc_sb = const.tile([128, 3, C], BF16, tag="wvc")
        wqt_sb = const.tile([128, 2, C], F32R, tag="wqt")
        wot_sb = const.tile([128, 2, C], F32R, tag="wot")
        srcf_sb = const.tile([128, 2, QCHUNK], F32, tag="srcf")
        srcr_sb = const.tile([128, 2, QCHUNK], F32R, tag="srcr")
        bq2_sb = const.tile([128, 2], F32, tag="bq2")
        bk2_sb = const.tile([128, 2], F32, tag="bk2")
        boe_sb = const.tile([128, 2], F32, tag="boe")
        ones_sb = const.tile([128, 32], BF16, tag="ones")
        # sync queue: feat half 1, then q-projection inputs
        nc.sync.dma_start(featv_sb[:, :, 0 : L // 2], featv[:, :, 0 : L // 2])
        nc.sync.dma_start(srcr_sb[:, :, 0:QN], srcq[:, :, 0:QN].bitcast(F32R))
        nc.sync.dma_start(srcr_sb[:, :, QN:], srcq[:, :, QN:].bitcast(F32R))
        nc.sync.dma_start(srcf_sb[:], srcq[:])
        # scalar queue: conv weights, q weights, feat half 2, the rest
        nc.scalar.dma_start(wkc_sb[:], wkc[:])
        nc.scalar.dma_start(wvc_sb[:], wvc[:])
        nc.scalar.dma_start(wqt_sb[:], wqt[:].bitcast(F32R))
        nc.scalar.dma_start(bq2_sb[:], bq2[:])
        nc.scalar.dma_start(bk2_sb[:], bk2[:])
        nc.scalar.dma_start(ones_sb[:], onesd[:])
        nc.scalar.dma_start(featv_sb[:, :, L // 2 :], featv[:, :, L // 2 :])
        nc.scalar.dma_start(wot_sb[:], wot[:].bitcast(F32R))
        nc.scalar.dma_start(boe_sb[:], boe[:])

        qT_sb = const.tile([128, 2, QCHUNK], BF16, tag="qT")
        kT_sb = const.tile([128, 2, L], BF16, tag="kT")
        v_sb = const.tile([128, KT, C], BF16, tag="v")

        # ---- Q^T = Wq @ src_chunk + bq   -> [C(part, 2 jo), QCHUNK] ----
        def emit_qproj(qn):
            for jo in range(2):
                ps = psc.tile([128, 1024], F32, tag="sc", name=f"q{jo}{qn}")[:, 0:QN]
                for ki in range(2):
                    nc.tensor.matmul(
                        ps[:],
                        wqt_sb[:, ki, jo * 128 : (jo + 1) * 128],
                        srcr_sb[:, ki, qn * QN : (qn + 1) * QN],
                        start=(ki == 0),
                        stop=(ki == 1),
                    )
                nc.vector.tensor_scalar_add(
                    qT_sb[:, jo, qn * QN : (qn + 1) * QN], ps[:], bq2_sb[:, jo : jo + 1]
                )

        # ---- conv emitters: kv block ln covers kv in [512ln, 512ln+512) ----
        def emit_conv_k(ln, jo):
            ps = psc.tile([128, 1024], F32, tag="sc", name=f"k{ln}{jo}")[:, 0:512]
            for kw in range(3):
                nc.tensor.matmul(
                    ps[:],
                    wkc_sb[0:FP, kw, jo * 128 : (jo + 1) * 128],
                    featv_sb[0:FP, kw, 512 * ln : 512 * (ln + 1)],
                    start=(kw == 0),
                    stop=(kw == 2),
                )
            nc.vector.tensor_scalar_add(
                kT_sb[:, jo, 512 * ln : 512 * (ln + 1)],
                ps[:],
                bk2_sb[:, jo : jo + 1],
            )

        def emit_conv_v(lt):
            ps = psc.tile([128, 1024], F32, tag="sc", name=f"v{lt}")[:, 0:QN]
            for kw in range(3):
                nc.tensor.matmul(
                    ps[:],
                    featv_sb[0:FP, kw, 128 * lt : 128 * (lt + 1)],
                    wvc_sb[0:FP, kw, :],
                    start=(kw == 0),
                    stop=(kw == 2),
                )
            nc.vector.tensor_copy(v_sb[:, lt, :], ps[:])

        # sequential conv pre-phase (kept out of the attention psc rotation:
        # sharing that rotation puts conv into the exp critical chain)
        for ln in range(8):
            for jo in range(2):
                emit_conv_k(ln, jo)
            for lt in range(4 * ln, 4 * ln + 4):
                emit_conv_v(lt)
        emit_qproj(0)

        # ---- attention: 4 q chunks x 32 kv tiles, software-pipelined ----
        def emit_scores_exp(qc, kt):
            p_tiles = []
            for t in range(2):
                sc = psc.tile([128, 1024], F32, tag="sc", name=f"s{qc}_{kt}_{t}")
                for g in (2 * t, 2 * t + 1):
                    for jo in range(2):
                        col = (2 * (g % 2) + jo) * QN
                        nc.tensor.matmul(
                            sc[:, col : col + QN],
                            kT_sb[32 * g : 32 * g + 32, jo, kt * 128 : (kt + 1) * 128],
                            qT_sb[32 * g : 32 * g + 32, jo, qc * QN : (qc + 1) * QN],
                            start=True,
                            stop=True,
                            tile_position=(32 * g, 0),
                            skip_group_check=True,
                        )
                p_sb = pwork.tile([128, 1024], BF16, tag="p", name=f"p{qc}_{kt}_{t}")
                nc.scalar.activation(p_sb[:], sc[:], AF.Exp, scale=SCALE)
                p_tiles.append(p_sb)
            return p_tiles

        def emit_pv(kt, p_tiles, u_t, d_t):
            for h in range(NH):
                g, jo = h % 4, h // 4
                psl = p_tiles[g // 2][:, (2 * (g % 2) + jo) * QN :][:, 0:QN]
                nc.tensor.matmul(
                    u_t[jo][32 * g : 32 * g + 32, 0:QN],
                    v_sb[:, kt, 32 * h : 32 * h + 32],
                    psl,
                    start=(kt == 0),
                    stop=(kt == KT - 1),
                    tile_position=(0, 32 * g),
                    skip_group_check=True,
                )
                nc.tensor.matmul(
                    d_t[jo][32 * g : 32 * g + 32, :],
                    ones_sb[:, 0:32],
                    psl,
                    start=(kt == 0),
                    stop=(kt == KT - 1),
                    tile_position=(0, 32 * g),
                    skip_group_check=True,
                )

        def emit_epilogue(qc, u_t, d_t):
            # normalize: o = U * (1/D) (D broadcast across each group's rows)
            rec_sb = work.tile([128, 2, QN], F32, tag="rec", name=f"rec{qc}")
            o_sb = work.tile([128, 2, QN], F32R, tag="o", name=f"o{qc}")
            for jo in range(2):
                nc.vector.reciprocal(rec_sb[:, jo, :], d_t[jo][:])
                nc.vector.tensor_tensor(
                    o_sb[:, jo, :], u_t[jo][:, 0:QN], rec_sb[:, jo, :], ALU.mult
                )
            # out projection + bias + * src, then store
            for jo in range(2):
                op = psc.tile([128, 1024], F32, tag="sc", name=f"op{qc}_{jo}")[:, 0:QN]
                for ki in range(2):
                    nc.tensor.matmul(
                        op[:],
                        wot_sb[:, ki, jo * 128 : (jo + 1) * 128],
                        o_sb[:, ki, :],
                        start=(ki == 0),
                        stop=(ki == 1),
                    )
                ot = work.tile([128, QN], F32, tag="ot", name=f"ot{qc}_{jo}")
                nc.vector.scalar_tensor_tensor(
                    ot[:],
                    op[:],
                    boe_sb[:, jo : jo + 1],
                    srcf_sb[:, jo, qc * QN : (qc + 1) * QN],
                    op0=ALU.add,
                    op1=ALU.mult,
                )
                nc.gpsimd.dma_start(
                    outq[jo * 128 : (jo + 1) * 128, qc * QN : (qc + 1) * QN], ot[:]
                )

        prev = None      # (kt, p_tiles, u_t, d_t) pending PV
        prev_epi = None  # (qc, u_t, d_t) pending epilogue
        for qc in range(NQC):
            u_t = [
                pacc.tile([128, 512], F32, tag="u", name=f"u{qc}_{jo}")
                for jo in range(2)
            ]
            d_t = [
                pacc.tile([128, 512], F32, tag="u", name=f"d{qc}_{jo}")[:, 0:QN]
                for jo in range(2)
            ]
            for kt in range(KT):
                p_tiles = emit_scores_exp(qc, kt)
                if prev is not None:
                    emit_pv(*prev)
                if prev_epi is not None:
                    emit_epilogue(*prev_epi)
                    prev_epi = None
                prev = (kt, p_tiles, u_t, d_t)
                if kt == 16 and qc < NQC - 1:
                    emit_qproj(qc + 1)
            prev_epi = (qc, u_t, d_t)
        emit_pv(*prev)
        emit_epilogue(*prev_epi)

    return nc


_CACHE: dict = {}


def _split_matmul_waits(nc: bass.Bass):
    """walrus's fp32r self-loading matmul (S3 LW struct) accepts only one
    sync-wait command; peel extra waits onto PE EventSemaphore ops inserted
    immediately before the matmul (same sync point, so no deadlock risk)."""
    import bass_rust

    n_new = 0
    for fn in nc.m.functions:
        for block in fn.blocks:
            insts = list(block.instructions)
            out = []
            changed = False
            skip = (
                mybir.InstEventSemaphore,
                mybir.InstAllEngineBarrier,
                mybir.InstHalt,
            )
            for inst in insts:
                if not isinstance(inst, skip) and inst.sync_info is not None:
                    si = inst.sync_info
                    waits = list(si.on_wait)
                    if len(waits) > 1:
                        for w in waits[:-1]:
                            ev = mybir.InstEventSemaphore(
                                name=f"WSPLIT-{n_new}", ins=[], outs=[]
                            )
                            ev.engine = inst.engine
                            ev.sync_info = bass_rust.SyncInfo(
                                on_wait=[w], on_update=[]
                            )
                            out.append(ev)
                            n_new += 1
                        inst.sync_info = bass_rust.SyncInfo(
                            on_wait=[waits[-1]], on_update=list(si.on_update)
                        )
                        changed = True
                out.append(inst)
            if changed:
                block.instructions = out
    return n_new


def get_nc() -> bass.Bass:
    if "nc" not in _CACHE:
        nc = bass.Bass()
        build_kernel(nc)
        _split_matmul_waits(nc)
        nc.finalize()
        _CACHE["nc"] = nc
    return _CACHE["nc"]


def make_core_inputs(feat, src, Wq, bq, Wk, bk, Wv, bv, Wo, bo):
    """Host-side sharding / layout prep. Returns list of 8 input dicts."""
    import ml_dtypes

    f32 = np.float32
    bf16 = ml_dtypes.bfloat16
    feat = np.asarray(feat, f32)
    src = np.asarray(src, f32)
    Wq, Wk, Wv, Wo = (np.asarray(x, f32) for x in (Wq, Wk, Wv, Wo))
    bq, bk, bv, bo = (np.asarray(x, f32) for x in (bq, bk, bv, bo))

    wqt = np.ascontiguousarray(Wq.T.reshape(2, 128, C).transpose(1, 0, 2))
    wot = np.ascontiguousarray(Wo.T.reshape(2, 128, C).transpose(1, 0, 2))

    # conv-tap layouts: wkc[kh*29+c, kw, cout] = Wk[cout, 9c+3kh+kw] (0 pad)
    wkc = np.zeros((128, 3, C), f32)
    wvc = np.zeros((128, 3, C), f32)
    for kh in range(3):
        for kw in range(3):
            for c in range(CF):
                j = 9 * c + 3 * kh + kw
                if j < C:
                    wkc[kh * CF + c, kw, :] = Wk[:, j]
                    wvc[kh * CF + c, kw, :] = Wv[:, j]
    wkc = wkc.astype(bf16)
    wvc = wvc.astype(bf16)
    onesd = np.ones((128, 32), bf16)

    bq2 = np.ascontiguousarray(bq.reshape(2, 128).T)
    bk2 = np.ascontiguousarray(bk.reshape(2, 128).T)
    boev = Wo @ bv + bo
    boe = np.ascontiguousarray(boev.reshape(2, 128).T)

    shared = dict(
        wqt=wqt, wot=wot, wkc=wkc, wvc=wvc, bq2=bq2, bk2=bk2, boe=boe, onesd=onesd
    )

    # featv[kh*29+c, kw, oh*64+ow] = feat[b, c, 2oh+kh-1, 2ow+kw-1], 0 OOB
    featv_b = []
    for b in range(B):
        fp = np.pad(feat[b, :CF], ((0, 0), (1, 1), (1, 1)))
        fv = np.zeros((128, 3, L), f32)
        for kh in range(3):
            for kw in range(3):
                patch = fp[:, kh : kh + 2 * H : 2, kw : kw + 2 * W : 2]
                fv[kh * CF : kh * CF + CF, kw, :] = patch.reshape(CF, L)
        featv_b.append(fv.astype(bf16))

    in_maps = []
    for core in range(NCORE):
        b, qi = divmod(core, 4)
        m = dict(shared)
        m["featv"] = featv_b[b]
        sl = src[b].reshape(C, L)[:, qi * QCHUNK : (qi + 1) * QCHUNK]
        m["srcq"] = np.ascontiguousarray(
            sl.reshape(2, 128, QCHUNK).transpose(1, 0, 2)
        )
        in_maps.append(m)
    return in_maps


def _ensure_ntff_hook():
    """Provide antenv.axon_hooks if the image lacks it (needed for trace=True)."""
    import contextlib
    import ctypes
    import os
    import sys
    import types

    try:
        import antenv.axon_hooks  # noqa: F401

        return
    except ImportError:
        pass

    mod = types.ModuleType("antenv.axon_hooks")
    box = [None]
    mod.set_axon_ntff_profile_hook = lambda h: box.__setitem__(0, h)
    mod.get_axon_ntff_profile_hook = lambda: box[0]
    sys.modules["antenv.axon_hooks"] = mod
    import antenv

    antenv.axon_hooks = mod

    so_path = os.environ.get("PJRT_LIBRARY_PATH", "/opt/axon/libaxon_pjrt.so")
    try:
        lib = ctypes.CDLL(so_path)
    except OSError:
        return
    if not hasattr(lib, "axon_start_nrt_profile"):
        return
    lib.axon_start_nrt_profile.argtypes = [
        ctypes.POINTER(ctypes.c_int64),
        ctypes.c_size_t,
    ]
    lib.axon_start_nrt_profile.restype = ctypes.c_int64
    lib.axon_stop_nrt_profile.argtypes = [ctypes.c_char_p]
    lib.axon_stop_nrt_profile.restype = ctypes.c_int64

    @contextlib.contextmanager
    def _hook(output_dir, device_ids):
        import jax

        jax.devices()
        if device_ids:
            ids = (ctypes.c_int64 * len(device_ids))(*device_ids)
            rc = lib.axon_start_nrt_profile(ids, len(device_ids))
        else:
            rc = lib.axon_start_nrt_profile(None, 0)
        if rc != 0:
            raise RuntimeError(f"axon_start_nrt_profile rc={rc}")
        try:
            yield
        finally:
            n = lib.axon_stop_nrt_profile(str(output_dir).encode())
            print(f"profile: {n} file(s) written to {output_dir}", file=sys.stderr)

    box[0] = _hook


def run(inputs: dict, trace: bool = False, trace_cores=None):
    _ensure_ntff_hook()
    from concourse.bass_utils import run_bass_kernel_spmd

    nc = get_nc()
    in_maps = make_core_inputs(**inputs)
    res = run_bass_kernel_spmd(
        nc,
        in_maps,
        list(range(NCORE)),
        trace=trace,
        trace_cores=trace_cores,
    )
    out = np.empty((B, C, L), np.float32)
    for core in range(NCORE):
        b, qi = divmod(core, 4)
        out[b, :, qi * QCHUNK : (qi + 1) * QCHUNK] = res.results[core]["outq"]
    return out.reshape(B, C, H, W), res


def kernel(feat, src, Wq, bq, Wk, bk, Wv, bv, Wo, bo):
    out, _ = run(
        dict(feat=feat, src=src, Wq=Wq, bq=bq, Wk=Wk, bk=bk, Wv=Wv, bv=bv, Wo=Wo, bo=bo)
    )
    return out


# revision 18
# speedup vs baseline: 2.3082x; 1.0276x over previous
"""Trainium2 Bass kernel for the CSSAM sparse-attention module (v2).

Math (per batch b):
  q_in  = src[b] viewed as [C, L] (L = 64*64 = 4096)               (queries)
  kv[j, l] = featpad[b, j//9, kh + 2*oh - 1, kw + 2*ow - 1]
             where (kh, kw) = divmod(j % 9, 3), l = oh*64 + ow     (keys/vals)
      -> only feat channels 0..28 are ever used
  Q^T = Wq @ q_in + bq ; K^T = Wk @ kv + bk ; V = kv^T Wv^T
  per head h (8 heads, d = 32): softmax((Qh^T)^T Kh / sqrt(d)) Vh
  out[b] = (Wo @ O^T + (Wo bv + bo)) * src[b]

Sharding: 8 cores = 2 batches x 4 query-chunks of 1024; K/V work is
replicated across the 4 cores of a batch.

v2 design notes (HW-model driven):
 - The kernel is Activation-engine bound: exp over 8 heads x 4096 kv x
   1024 q per core = 262144 rows/partition ~ 218 us busy minimum. All
   other engines are paced to hide under it.
 - Scores matmuls use 4-way row-group concurrency (tile_position=(32g,0));
   PV + denominator use 4-way col-group concurrency (tile_position=(0,32g)).
 - Denominator rows are broadcast to all 32 partitions of a group by a
   [128, 32] ones lhsT (M=32), so normalization is a plain elementwise
   multiply by 1/D. D accumulates in its own PSUM bank - interleaving the
   D accumulation group into the U bank corrupts U on hardware.
 - K/V convs pack (channel, kh) into 87 partitions via a host-prepared,
   per-tap pre-shifted im2col layout (featv), so each conv output needs
   only 3 accumulating matmuls (kw taps) and both convs are plain 2D.
 - Software pipelining: scores for kv-tile kt+1 are emitted before PV of
   kt so the PE never serializes behind the Activation engine.

PSUM budget (8 banks x 2KB, bank-granular): sc 2x[128,1024] = 4,
U/D 4x[128,512] = 4 (a full bank per accumulator - an open accumulation
group must not share a bank with any other matmul group, or it loses
contributions on hardware). Conv/proj psum borrows the sc rotation.
"""

from contextlib import ExitStack

import numpy as np

import concourse.bass as bass
import concourse.mybir as mybir
import concourse.tile as tile

F32 = mybir.dt.float32
F32R = mybir.dt.float32r
BF16 = mybir.dt.bfloat16
AF = mybir.ActivationFunctionType
ALU = mybir.AluOpType

B = 2
C = 256
NH = 8
HD = 32
H = W = 64
L = H * W            # 4096 query / kv positions per batch
HF = WF = 128        # feat spatial
CF = 29              # feat channels actually used by the module
NCORE = 8
QCHUNK = L // 4      # 1024 queries per core
QN = 256             # attention q sub-chunk
NQC = QCHUNK // QN   # 4
KT = L // 128        # 32 kv tiles of 128
SCALE = float(1.0 / np.sqrt(HD))
FP = 87              # conv partitions: 3 kh taps x 29 channels


def build_kernel(nc: bass.Bass):
    # featv[kh*29+c, kw, oh*64+ow] = feat[c, 2*oh + kh - 1, 2*ow + kw - 1]
    # (0 out of bounds): per-tap pre-shifted im2col planes, host-prepared
    featv = nc.declare_dram_parameter("featv", [128, 3, L], BF16, isOutput=False)
    srcq = nc.declare_dram_parameter("srcq", [128, 2, QCHUNK], F32, isOutput=False)
    wqt = nc.declare_dram_parameter("wqt", [128, 2, C], F32, isOutput=False)
    wot = nc.declare_dram_parameter("wot", [128, 2, C], F32, isOutput=False)
    wkc = nc.declare_dram_parameter("wkc", [128, 3, C], BF16, isOutput=False)
    wvc = nc.declare_dram_parameter("wvc", [128, 3, C], BF16, isOutput=False)
    bq2 = nc.declare_dram_parameter("bq2", [128, 2], F32, isOutput=False)
    bk2 = nc.declare_dram_parameter("bk2", [128, 2], F32, isOutput=False)
    boe = nc.declare_dram_parameter("boe", [128, 2], F32, isOutput=False)
    onesd = nc.declare_dram_parameter("onesd", [128, 32], BF16, isOutput=False)
    outq = nc.declare_dram_parameter("outq", [C, QCHUNK], F32, isOutput=True)

    with ExitStack() as ctx:
        ctx.enter_context(
            nc.allow_low_precision("float32r tiles carry full fp32 bits")
        )
        tc = ctx.enter_context(tile.TileContext(nc))
        const = ctx.enter_context(tc.tile_pool(name="const", bufs=1))
        work = ctx.enter_context(tc.tile_pool(name="work", bufs=2))
        pwork = ctx.enter_context(tc.tile_pool(name="pwork", bufs=4))
        psc = ctx.enter_context(tc.tile_pool(name="psc", bufs=2, space="PSUM"))
        pacc = ctx.enter_context(tc.tile_pool(name="pacc", bufs=4, space="PSUM"))

        # ---- exp table preload (hide the 1.3us ACT_TABLE_LOAD under DMAs) ----
        dummy = work.tile([128, 1], F32, tag="dummy", name="dummy")
        nc.gpsimd.memset(dummy[:], 0.0)
        nc.scalar.activation(dummy[:], dummy[:], AF.Exp)

        # ---- input loads, critical path first, split across DMA queues ----
        featv_sb = const.tile([128, 3, L], BF16, tag="featv")
        wkc_sb = const.tile([128, 3, C], BF16, tag="wkc")
        wvc_sb = const.tile([128, 3, C], BF16, tag="wvc")
        wqt_sb = const.tile([128, 2, C], F32R, tag="wqt")
        wot_sb = const.tile([128, 2, C], F32R, tag="wot")
        srcf_sb = const.tile([128, 2, QCHUNK], F32, tag="srcf")
        srcr_sb = const.tile([128, 2, QCHUNK], F32R, tag="srcr")
        bq2_sb = const.tile([128, 2], F32, tag="bq2")
        bk2_sb = const.tile([128, 2], F32, tag="bk2")
        boe_sb = const.tile([128, 2], F32, tag="boe")
        ones_sb = const.tile([128, 32], BF16, tag="ones")
        # sync queue: feat half 1, then q-projection inputs
        nc.sync.dma_start(featv_sb[:, :, 0 : L // 2], featv[:, :, 0 : L // 2])
        nc.sync.dma_start(srcr_sb[:, :, 0:QN], srcq[:, :, 0:QN].bitcast(F32R))
        nc.sync.dma_start(srcr_sb[:, :, QN:], srcq[:, :, QN:].bitcast(F32R))
        nc.sync.dma_start(srcf_sb[:], srcq[:])
        # scalar queue: conv weights, q weights, feat half 2, the rest
        nc.scalar.dma_start(wkc_sb[:], wkc[:])
        nc.scalar.dma_start(wvc_sb[:], wvc[:])
        nc.scalar.dma_start(wqt_sb[:], wqt[:].bitcast(F32R))
        nc.scalar.dma_start(bq2_sb[:], bq2[:])
        nc.scalar.dma_start(bk2_sb[:], bk2[:])
        nc.scalar.dma_start(ones_sb[:], onesd[:])
        nc.scalar.dma_start(featv_sb[:, :, L // 2 :], featv[:, :, L // 2 :])
        nc.scalar.dma_start(wot_sb[:], wot[:].bitcast(F32R))
        nc.scalar.dma_start(boe_sb[:], boe[:])

        qT_sb = const.tile([128, 2, QCHUNK], BF16, tag="qT")
        kT_sb = const.tile([128, 2, L], BF16, tag="kT")
        v_sb = const.tile([128, KT, C], BF16, tag="v")

        # ---- Q^T = Wq @ src_chunk + bq   -> [C(part, 2 jo), QCHUNK] ----
        def emit_qproj(qn):
            for jo in range(2):
                ps = pacc.tile([128, 512], F32, tag="u", name=f"q{jo}{qn}")[:, 0:QN]
                for ki in range(2):
                    nc.tensor.matmul(
                        ps[:],
                        wqt_sb[:, ki, jo * 128 : (jo + 1) * 128],
                        srcr_sb[:, ki, qn * QN : (qn + 1) * QN],
                        start=(ki == 0),
                        stop=(ki == 1),
                    )
                nc.vector.tensor_scalar_add(
                    qT_sb[:, jo, qn * QN : (qn + 1) * QN], ps[:], bq2_sb[:, jo : jo + 1]
                )

        # ---- conv emitters: kv block ln covers kv in [512ln, 512ln+512) ----
        def emit_conv_k(ln, jo):
            ps = pacc.tile([128, 512], F32, tag="u", name=f"k{ln}{jo}")
            for kw in range(3):
                nc.tensor.matmul(
                    ps[:],
                    wkc_sb[0:FP, kw, jo * 128 : (jo + 1) * 128],
                    featv_sb[0:FP, kw, 512 * ln : 512 * (ln + 1)],
                    start=(kw == 0),
                    stop=(kw == 2),
                )
            nc.vector.tensor_scalar_add(
                kT_sb[:, jo, 512 * ln : 512 * (ln + 1)],
                ps[:],
                bk2_sb[:, jo : jo + 1],
            )

        def emit_conv_v(lt):
            ps = pacc.tile([128, 512], F32, tag="u", name=f"v{lt}")[:, 0:QN]
            for kw in range(3):
                nc.tensor.matmul(
                    ps[:],
                    featv_sb[0:FP, kw, 128 * lt : 128 * (lt + 1)],
                    wvc_sb[0:FP, kw, :],
                    start=(kw == 0),
                    stop=(kw == 2),
                )
            nc.vector.tensor_copy(v_sb[:, lt, :], ps[:])

        # sequential conv pre-phase. All pre-phase psum (q-proj, conv) lives
        # in the pacc rotation (4 banks, idle until attention) so the sc
        # rotation never pulls conv into the exp critical chain.
        emit_qproj(0)
        for ln in range(8):
            for jo in range(2):
                emit_conv_k(ln, jo)
            for lt in range(4 * ln, 4 * ln + 4):
                emit_conv_v(lt)
        for qn in range(1, 4):
            emit_qproj(qn)

        # ---- attention: 4 q chunks x 32 kv tiles, software-pipelined ----
        def emit_scores_exp(qc, kt):
            p_tiles = []
            for t in range(2):
                sc = psc.tile([128, 1024], F32, tag="sc", name=f"s{qc}_{kt}_{t}")
                for g in (2 * t, 2 * t + 1):
                    for jo in range(2):
                        col = (2 * (g % 2) + jo) * QN
                        nc.tensor.matmul(
                            sc[:, col : col + QN],
                            kT_sb[32 * g : 32 * g + 32, jo, kt * 128 : (kt + 1) * 128],
                            qT_sb[32 * g : 32 * g + 32, jo, qc * QN : (qc + 1) * QN],
                            start=True,
                            stop=True,
                            tile_position=(32 * g, 0),
                            skip_group_check=True,
                        )
                p_sb = pwork.tile([128, 1024], BF16, tag="p", name=f"p{qc}_{kt}_{t}")
                nc.scalar.activation(p_sb[:], sc[:], AF.Exp, scale=SCALE)
                p_tiles.append(p_sb)
            return p_tiles

        def emit_pv(kt, p_tiles, u_t, d_t):
            for h in range(NH):
                g, jo = h % 4, h // 4
                psl = p_tiles[g // 2][:, (2 * (g % 2) + jo) * QN :][:, 0:QN]
                nc.tensor.matmul(
                    u_t[jo][32 * g : 32 * g + 32, 0:QN],
                    v_sb[:, kt, 32 * h : 32 * h + 32],
                    psl,
                    start=(kt == 0),
                    stop=(kt == KT - 1),
                    tile_position=(0, 32 * g),
                    skip_group_check=True,
                )
                nc.tensor.matmul(
                    d_t[jo][32 * g : 32 * g + 32, :],
                    ones_sb[:, 0:32],
                    psl,
                    start=(kt == 0),
                    stop=(kt == KT - 1),
                    tile_position=(0, 32 * g),
                    skip_group_check=True,
                )

        def emit_epilogue(qc, u_t, d_t):
            # normalize: o = U * (1/D) (D broadcast across each group's rows)
            rec_sb = work.tile([128, 2, QN], F32, tag="rec", name=f"rec{qc}")
            o_sb = work.tile([128, 2, QN], F32R, tag="o", name=f"o{qc}")
            for jo in range(2):
                nc.vector.reciprocal(rec_sb[:, jo, :], d_t[jo][:])
                nc.vector.tensor_tensor(
                    o_sb[:, jo, :], u_t[jo][:, 0:QN], rec_sb[:, jo, :], ALU.mult
                )
            # out projection + bias + * src, then store
            for jo in range(2):
                op = pacc.tile([128, 512], F32, tag="u", name=f"op{qc}_{jo}")[:, 0:QN]
                for ki in range(2):
                    nc.tensor.matmul(
                        op[:],
                        wot_sb[:, ki, jo * 128 : (jo + 1) * 128],
                        o_sb[:, ki, :],
                        start=(ki == 0),
                        stop=(ki == 1),
                    )
                ot = work.tile([128, QN], F32, tag="ot", name=f"ot{qc}_{jo}")
                nc.vector.scalar_tensor_tensor(
                    ot[:],
                    op[:],
                    boe_sb[:, jo : jo + 1],
                    srcf_sb[:, jo, qc * QN : (qc + 1) * QN],
                    op0=ALU.add,
                    op1=ALU.mult,
                )
                nc.gpsimd.dma_start(
                    outq[jo * 128 : (jo + 1) * 128, qc * QN : (qc + 1) * QN], ot[:]
                )

        prev = None      # (kt, p_tiles, u_t, d_t) pending PV
        prev_epi = None  # (qc, u_t, d_t) pending epilogue
        for qc in range(NQC):
            u_t = d_t = None
            for kt in range(KT):
                p_tiles = emit_scores_exp(qc, kt)
                if prev is not None:
                    emit_pv(*prev)
                if prev_epi is not None:
                    emit_epilogue(*prev_epi)
                    prev_epi = None
                if kt == 0:
                    # allocated after the previous epilogue's op tiles so the
                    # pacc rotation frees banks in dependency order
                    u_t = [
                        pacc.tile([128, 512], F32, tag="u", name=f"u{qc}_{jo}")
                        for jo in range(2)
                    ]
                    d_t = [
                        pacc.tile([128, 512], F32, tag="u", name=f"d{qc}_{jo}")[
                            :, 0:QN
                        ]
                        for jo in range(2)
                    ]
                prev = (kt, p_tiles, u_t, d_t)
            prev_epi = (qc, u_t, d_t)
        emit_pv(*prev)
        emit_epilogue(*prev_epi)

    return nc


_CACHE: dict = {}


def _split_matmul_waits(nc: bass.Bass):
    """walrus's fp32r self-loading matmul (S3 LW struct) accepts only one
    sync-wait command; peel extra waits onto PE EventSemaphore ops inserted
    immediately before the matmul (same sync point, so no deadlock risk)."""
    import bass_rust

    n_new = 0
    for fn in nc.m.functions:
        for block in fn.blocks:
            insts = list(block.instructions)
            out = []
            changed = False
            skip = (
                mybir.InstEventSemaphore,
                mybir.InstAllEngineBarrier,
                mybir.InstHalt,
            )
            for inst in insts:
                if not isinstance(inst, skip) and inst.sync_info is not None:
                    si = inst.sync_info
                    waits = list(si.on_wait)
                    if len(waits) > 1:
                        for w in waits[:-1]:
                            ev = mybir.InstEventSemaphore(
                                name=f"WSPLIT-{n_new}", ins=[], outs=[]
                            )
                            ev.engine = inst.engine
                            ev.sync_info = bass_rust.SyncInfo(
                                on_wait=[w], on_update=[]
                            )
                            out.append(ev)
                            n_new += 1
                        inst.sync_info = bass_rust.SyncInfo(
                            on_wait=[waits[-1]], on_update=list(si.on_update)
                        )
                        changed = True
                out.append(inst)
            if changed:
                block.instructions = out
    return n_new


def get_nc() -> bass.Bass:
    if "nc" not in _CACHE:
        nc = bass.Bass()
        build_kernel(nc)
        _split_matmul_waits(nc)
        nc.finalize()
        _CACHE["nc"] = nc
    return _CACHE["nc"]


def make_core_inputs(feat, src, Wq, bq, Wk, bk, Wv, bv, Wo, bo):
    """Host-side sharding / layout prep. Returns list of 8 input dicts."""
    import ml_dtypes

    f32 = np.float32
    bf16 = ml_dtypes.bfloat16
    feat = np.asarray(feat, f32)
    src = np.asarray(src, f32)
    Wq, Wk, Wv, Wo = (np.asarray(x, f32) for x in (Wq, Wk, Wv, Wo))
    bq, bk, bv, bo = (np.asarray(x, f32) for x in (bq, bk, bv, bo))

    wqt = np.ascontiguousarray(Wq.T.reshape(2, 128, C).transpose(1, 0, 2))
    wot = np.ascontiguousarray(Wo.T.reshape(2, 128, C).transpose(1, 0, 2))

    # conv-tap layouts: wkc[kh*29+c, kw, cout] = Wk[cout, 9c+3kh+kw] (0 pad)
    wkc = np.zeros((128, 3, C), f32)
    wvc = np.zeros((128, 3, C), f32)
    for kh in range(3):
        for kw in range(3):
            for c in range(CF):
                j = 9 * c + 3 * kh + kw
                if j < C:
                    wkc[kh * CF + c, kw, :] = Wk[:, j]
                    wvc[kh * CF + c, kw, :] = Wv[:, j]
    wkc = wkc.astype(bf16)
    wvc = wvc.astype(bf16)
    onesd = np.ones((128, 32), bf16)

    bq2 = np.ascontiguousarray(bq.reshape(2, 128).T)
    bk2 = np.ascontiguousarray(bk.reshape(2, 128).T)
    boev = Wo @ bv + bo
    boe = np.ascontiguousarray(boev.reshape(2, 128).T)

    shared = dict(
        wqt=wqt, wot=wot, wkc=wkc, wvc=wvc, bq2=bq2, bk2=bk2, boe=boe, onesd=onesd
    )

    # featv[kh*29+c, kw, oh*64+ow] = feat[b, c, 2oh+kh-1, 2ow+kw-1], 0 OOB
    featv_b = []
    for b in range(B):
        fp = np.pad(feat[b, :CF], ((0, 0), (1, 1), (1, 1)))
        fv = np.zeros((128, 3, L), f32)
        for kh in range(3):
            for kw in range(3):
                patch = fp[:, kh : kh + 2 * H : 2, kw : kw + 2 * W : 2]
                fv[kh * CF : kh * CF + CF, kw, :] = patch.reshape(CF, L)
        featv_b.append(fv.astype(bf16))

    in_maps = []
    for core in range(NCORE):
        b, qi = divmod(core, 4)
        m = dict(shared)
        m["featv"] = featv_b[b]
        sl = src[b].reshape(C, L)[:, qi * QCHUNK : (qi + 1) * QCHUNK]
        m["srcq"] = np.ascontiguousarray(
            sl.reshape(2, 128, QCHUNK).transpose(1, 0, 2)
        )
        in_maps.append(m)
    return in_maps


def _ensure_ntff_hook():
    """Provide antenv.axon_hooks if the image lacks it (needed for trace=True)."""
    import contextlib
    import ctypes
    import os
    import sys
    import types

    try:
        import antenv.axon_hooks  # noqa: F401

        return
    except ImportError:
        pass

    mod = types.ModuleType("antenv.axon_hooks")
    box = [None]
    mod.set_axon_ntff_profile_hook = lambda h: box.__setitem__(0, h)
    mod.get_axon_ntff_profile_hook = lambda: box[0]
    sys.modules["antenv.axon_hooks"] = mod
    import antenv

    antenv.axon_hooks = mod

    so_path = os.environ.get("PJRT_LIBRARY_PATH", "/opt/axon/libaxon_pjrt.so")
    try:
        lib = ctypes.CDLL(so_path)
    except OSError:
        return
    if not hasattr(lib, "axon_start_nrt_profile"):
        return
    lib.axon_start_nrt_profile.argtypes = [
        ctypes.POINTER(ctypes.c_int64),
        ctypes.c_size_t,
    ]
    lib.axon_start_nrt_profile.restype = ctypes.c_int64
    lib.axon_stop_nrt_profile.argtypes = [ctypes.c_char_p]
    lib.axon_stop_nrt_profile.restype = ctypes.c_int64

    @contextlib.contextmanager
    def _hook(output_dir, device_ids):
        import jax

        jax.devices()
        if device_ids:
            ids = (ctypes.c_int64 * len(device_ids))(*device_ids)
            rc = lib.axon_start_nrt_profile(ids, len(device_ids))
        else:
            rc = lib.axon_start_nrt_profile(None, 0)
        if rc != 0:
            raise RuntimeError(f"axon_start_nrt_profile rc={rc}")
        try:
            yield
        finally:
            n = lib.axon_stop_nrt_profile(str(output_dir).encode())
            print(f"profile: {n} file(s) written to {output_dir}", file=sys.stderr)

    box[0] = _hook


def run(inputs: dict, trace: bool = False, trace_cores=None):
    _ensure_ntff_hook()
    from concourse.bass_utils import run_bass_kernel_spmd

    nc = get_nc()
    in_maps = make_core_inputs(**inputs)
    res = run_bass_kernel_spmd(
        nc,
        in_maps,
        list(range(NCORE)),
        trace=trace,
        trace_cores=trace_cores,
    )
    out = np.empty((B, C, L), np.float32)
    for core in range(NCORE):
        b, qi = divmod(core, 4)
        out[b, :, qi * QCHUNK : (qi + 1) * QCHUNK] = res.results[core]["outq"]
    return out.reshape(B, C, H, W), res


def kernel(feat, src, Wq, bq, Wk, bk, Wv, bv, Wo, bo):
    out, _ = run(
        dict(feat=feat, src=src, Wq=Wq, bq=bq, Wk=Wk, bk=bk, Wv=Wv, bv=bv, Wo=Wo, bo=bo)
    )
    return out


# revision 20
# speedup vs baseline: 2.3763x; 1.0295x over previous
"""Trainium2 Bass kernel for the CSSAM sparse-attention module (v2).

Math (per batch b):
  q_in  = src[b] viewed as [C, L] (L = 64*64 = 4096)               (queries)
  kv[j, l] = featpad[b, j//9, kh + 2*oh - 1, kw + 2*ow - 1]
             where (kh, kw) = divmod(j % 9, 3), l = oh*64 + ow     (keys/vals)
      -> only feat channels 0..28 are ever used
  Q^T = Wq @ q_in + bq ; K^T = Wk @ kv + bk ; V = kv^T Wv^T
  per head h (8 heads, d = 32): softmax((Qh^T)^T Kh / sqrt(d)) Vh
  out[b] = (Wo @ O^T + (Wo bv + bo)) * src[b]

Sharding: 8 cores = 2 batches x 4 query-chunks of 1024; K/V work is
replicated across the 4 cores of a batch.

v2 design notes (HW-model driven):
 - The kernel is Activation-engine bound: exp over 8 heads x 4096 kv x
   1024 q per core = 262144 rows/partition ~ 218 us busy minimum. All
   other engines are paced to hide under it.
 - Scores matmuls use 4-way row-group concurrency (tile_position=(32g,0));
   PV + denominator use 4-way col-group concurrency (tile_position=(0,32g)).
 - Denominator rows are broadcast to all 32 partitions of a group by a
   [128, 32] ones lhsT (M=32), so normalization is a plain elementwise
   multiply by 1/D. D accumulates in its own PSUM bank - interleaving the
   D accumulation group into the U bank corrupts U on hardware.
 - K/V convs pack (channel, kh) into 87 partitions via a host-prepared,
   per-tap pre-shifted im2col layout (featv), so each conv output needs
   only 3 accumulating matmuls (kw taps) and both convs are plain 2D.
 - Software pipelining: scores for kv-tile kt+1 are emitted before PV of
   kt so the PE never serializes behind the Activation engine.

PSUM budget (8 banks x 2KB, bank-granular): sc 2x[128,1024] = 4,
U/D 4x[128,512] = 4 (a full bank per accumulator - an open accumulation
group must not share a bank with any other matmul group, or it loses
contributions on hardware). Conv/proj psum borrows the sc rotation.
"""

from contextlib import ExitStack

import numpy as np

import concourse.bass as bass
import concourse.mybir as mybir
import concourse.tile as tile

F32 = mybir.dt.float32
F32R = mybir.dt.float32r
BF16 = mybir.dt.bfloat16
AF = mybir.ActivationFunctionType
ALU = mybir.AluOpType

B = 2
C = 256
NH = 8
HD = 32
H = W = 64
L = H * W            # 4096 query / kv positions per batch
HF = WF = 128        # feat spatial
CF = 29              # feat channels actually used by the module
NCORE = 8
QCHUNK = L // 4      # 1024 queries per core
QN = 256             # attention q sub-chunk
NQC = QCHUNK // QN   # 4
KT = L // 128        # 32 kv tiles of 128
SCALE = float(1.0 / np.sqrt(HD))
FP = 87              # conv partitions: 3 kh taps x 29 channels


def build_kernel(nc: bass.Bass):
    # featv[kh*29+c, kw, oh*64+ow] = feat[c, 2*oh + kh - 1, 2*ow + kw - 1]
    # (0 out of bounds): per-tap pre-shifted im2col planes, host-prepared
    featv = nc.declare_dram_parameter("featv", [128, 3, L], BF16, isOutput=False)
    srcq = nc.declare_dram_parameter("srcq", [128, 2, QCHUNK], F32, isOutput=False)
    wqt = nc.declare_dram_parameter("wqt", [128, 2, C], F32, isOutput=False)
    wot = nc.declare_dram_parameter("wot", [128, 2, C], F32, isOutput=False)
    wkc = nc.declare_dram_parameter("wkc", [128, 3, C], BF16, isOutput=False)
    wvc = nc.declare_dram_parameter("wvc", [128, 3, C], BF16, isOutput=False)
    bq2 = nc.declare_dram_parameter("bq2", [128, 2], F32, isOutput=False)
    bk2 = nc.declare_dram_parameter("bk2", [128, 2], F32, isOutput=False)
    boe = nc.declare_dram_parameter("boe", [128, 2], F32, isOutput=False)
    onesd = nc.declare_dram_parameter("onesd", [128, 32], BF16, isOutput=False)
    outq = nc.declare_dram_parameter("outq", [C, QCHUNK], F32, isOutput=True)

    with ExitStack() as ctx:
        ctx.enter_context(
            nc.allow_low_precision("float32r tiles carry full fp32 bits")
        )
        tc = ctx.enter_context(tile.TileContext(nc))
        const = ctx.enter_context(tc.tile_pool(name="const", bufs=1))
        work = ctx.enter_context(tc.tile_pool(name="work", bufs=2))
        pwork = ctx.enter_context(tc.tile_pool(name="pwork", bufs=4))
        psc = ctx.enter_context(tc.tile_pool(name="psc", bufs=2, space="PSUM"))
        pacc = ctx.enter_context(tc.tile_pool(name="pacc", bufs=4, space="PSUM"))

        # ---- exp table preload (hide the 1.3us ACT_TABLE_LOAD under DMAs) ----
        dummy = work.tile([128, 1], F32, tag="dummy", name="dummy")
        nc.gpsimd.memset(dummy[:], 0.0)
        nc.scalar.activation(dummy[:], dummy[:], AF.Exp)

        # ---- input loads, critical path first, split across DMA queues ----
        featv_sb = const.tile([128, 3, L], BF16, tag="featv")
        wkc_sb = const.tile([128, 3, C], BF16, tag="wkc")
        wvc_sb = const.tile([128, 3, C], BF16, tag="wvc")
        wqt_sb = const.tile([128, 2, C], F32R, tag="wqt")
        wot_sb = const.tile([128, 2, C], F32R, tag="wot")
        srcf_sb = const.tile([128, 2, QCHUNK], F32, tag="srcf")
        srcr_sb = const.tile([128, 2, QCHUNK], F32R, tag="srcr")
        bq2_sb = const.tile([128, 2], F32, tag="bq2")
        bk2_sb = const.tile([128, 2], F32, tag="bk2")
        boe_sb = const.tile([128, 2], F32, tag="boe")
        ones_sb = const.tile([128, 32], BF16, tag="ones")
        # sync queue: feat chunk 0, q-projection inputs, feat chunk 2
        Q4 = L // 4
        nc.sync.dma_start(featv_sb[:, :, 0:Q4], featv[:, :, 0:Q4])
        nc.sync.dma_start(srcr_sb[:, :, 0:QN], srcq[:, :, 0:QN].bitcast(F32R))
        nc.sync.dma_start(featv_sb[:, :, Q4 : 2 * Q4], featv[:, :, Q4 : 2 * Q4])
        nc.sync.dma_start(srcr_sb[:, :, QN:], srcq[:, :, QN:].bitcast(F32R))
        nc.sync.dma_start(srcf_sb[:], srcq[:])
        # scalar queue: conv weights, q weights, feat chunks 1/3, the rest
        nc.scalar.dma_start(wkc_sb[:], wkc[:])
        nc.scalar.dma_start(wvc_sb[:], wvc[:])
        nc.scalar.dma_start(wqt_sb[:], wqt[:].bitcast(F32R))
        nc.scalar.dma_start(bq2_sb[:], bq2[:])
        nc.scalar.dma_start(bk2_sb[:], bk2[:])
        nc.scalar.dma_start(ones_sb[:], onesd[:])
        nc.scalar.dma_start(
            featv_sb[:, :, 2 * Q4 : 3 * Q4], featv[:, :, 2 * Q4 : 3 * Q4]
        )
        nc.scalar.dma_start(featv_sb[:, :, 3 * Q4 :], featv[:, :, 3 * Q4 :])
        nc.scalar.dma_start(wot_sb[:], wot[:].bitcast(F32R))
        nc.scalar.dma_start(boe_sb[:], boe[:])

        qT_sb = const.tile([128, 2, QCHUNK], BF16, tag="qT")
        kT_sb = const.tile([128, 2, L], BF16, tag="kT")
        v_sb = const.tile([128, KT, C], BF16, tag="v")

        # ---- Q^T = Wq @ src_chunk + bq   -> [C(part, 2 jo), QCHUNK] ----
        def emit_qproj(qn):
            for jo in range(2):
                ps = pacc.tile([128, 512], F32, tag="u", name=f"q{jo}{qn}")[:, 0:QN]
                for ki in range(2):
                    nc.tensor.matmul(
                        ps[:],
                        wqt_sb[:, ki, jo * 128 : (jo + 1) * 128],
                        srcr_sb[:, ki, qn * QN : (qn + 1) * QN],
                        start=(ki == 0),
                        stop=(ki == 1),
                    )
                nc.vector.tensor_scalar_add(
                    qT_sb[:, jo, qn * QN : (qn + 1) * QN], ps[:], bq2_sb[:, jo : jo + 1]
                )

        # ---- conv emitters: kv block ln covers kv in [512ln, 512ln+512) ----
        def emit_conv_k(ln, jo):
            ps = pacc.tile([128, 512], F32, tag="u", name=f"k{ln}{jo}")
            for kw in range(3):
                nc.tensor.matmul(
                    ps[:],
                    wkc_sb[0:FP, kw, jo * 128 : (jo + 1) * 128],
                    featv_sb[0:FP, kw, 512 * ln : 512 * (ln + 1)],
                    start=(kw == 0),
                    stop=(kw == 2),
                )
            # scalar engine is idle in the pre-phase; Identity is in every
            # act table so this cannot thrash the Exp table
            nc.scalar.activation(
                kT_sb[:, jo, 512 * ln : 512 * (ln + 1)],
                ps[:],
                AF.Identity,
                bias=bk2_sb[:, jo : jo + 1],
            )

        def emit_conv_v(lt):
            ps = pacc.tile([128, 512], F32, tag="u", name=f"v{lt}")[:, 0:QN]
            for kw in range(3):
                nc.tensor.matmul(
                    ps[:],
                    featv_sb[0:FP, kw, 128 * lt : 128 * (lt + 1)],
                    wvc_sb[0:FP, kw, :],
                    start=(kw == 0),
                    stop=(kw == 2),
                )
            nc.vector.tensor_copy(v_sb[:, lt, :], ps[:])

        # sequential conv pre-phase. All pre-phase psum (q-proj, conv) lives
        # in the pacc rotation (4 banks, idle until attention) so the sc
        # rotation never pulls conv into the exp critical chain.
        emit_qproj(0)
        for ln in range(8):
            for jo in range(2):
                emit_conv_k(ln, jo)
            for lt in range(4 * ln, 4 * ln + 4):
                emit_conv_v(lt)
            if ln in (1, 3, 5):
                emit_qproj(ln // 2 + 1)

        # ---- attention: 4 q chunks x 32 kv tiles, software-pipelined ----
        def emit_scores_exp(qc, kt):
            p_tiles = []
            for t in range(2):
                sc = psc.tile([128, 1024], F32, tag="sc", name=f"s{qc}_{kt}_{t}")
                for g in (2 * t, 2 * t + 1):
                    for jo in range(2):
                        col = (2 * (g % 2) + jo) * QN
                        nc.tensor.matmul(
                            sc[:, col : col + QN],
                            kT_sb[32 * g : 32 * g + 32, jo, kt * 128 : (kt + 1) * 128],
                            qT_sb[32 * g : 32 * g + 32, jo, qc * QN : (qc + 1) * QN],
                            start=True,
                            stop=True,
                            tile_position=(32 * g, 0),
                            skip_group_check=True,
                        )
                p_sb = pwork.tile([128, 1024], BF16, tag="p", name=f"p{qc}_{kt}_{t}")
                nc.scalar.activation(p_sb[:], sc[:], AF.Exp, scale=SCALE)
                p_tiles.append(p_sb)
            return p_tiles

        def emit_pv(kt, p_tiles, u_t, d_t):
            for h in range(NH):
                g, jo = h % 4, h // 4
                psl = p_tiles[g // 2][:, (2 * (g % 2) + jo) * QN :][:, 0:QN]
                nc.tensor.matmul(
                    u_t[jo][32 * g : 32 * g + 32, 0:QN],
                    v_sb[:, kt, 32 * h : 32 * h + 32],
                    psl,
                    start=(kt == 0),
                    stop=(kt == KT - 1),
                    tile_position=(0, 32 * g),
                    skip_group_check=True,
                )
                nc.tensor.matmul(
                    d_t[jo][32 * g : 32 * g + 32, :],
                    ones_sb[:, 0:32],
                    psl,
                    start=(kt == 0),
                    stop=(kt == KT - 1),
                    tile_position=(0, 32 * g),
                    skip_group_check=True,
                )

        def emit_norm(qc, u_t, d_t):
            # normalize: o = U * (1/D) (D broadcast across each group's rows)
            rec_sb = work.tile([128, 2, QN], F32, tag="rec", name=f"rec{qc}")
            o_sb = work.tile([128, 2, QN], F32R, tag="o", name=f"o{qc}")
            for jo in range(2):
                nc.vector.reciprocal(rec_sb[:, jo, :], d_t[jo][:])
                nc.vector.tensor_tensor(
                    o_sb[:, jo, :], u_t[jo][:, 0:QN], rec_sb[:, jo, :], ALU.mult
                )
            return o_sb

        def emit_oproj(qc, o_sb):
            # out projection + bias + * src, then store
            for jo in range(2):
                op = psc.tile([128, 1024], F32, tag="sc", name=f"op{qc}_{jo}")[
                    :, 0:QN
                ]
                for ki in range(2):
                    nc.tensor.matmul(
                        op[:],
                        wot_sb[:, ki, jo * 128 : (jo + 1) * 128],
                        o_sb[:, ki, :],
                        start=(ki == 0),
                        stop=(ki == 1),
                    )
                ot = work.tile([128, QN], F32, tag="ot", name=f"ot{qc}_{jo}")
                nc.vector.scalar_tensor_tensor(
                    ot[:],
                    op[:],
                    boe_sb[:, jo : jo + 1],
                    srcf_sb[:, jo, qc * QN : (qc + 1) * QN],
                    op0=ALU.add,
                    op1=ALU.mult,
                )
                nc.gpsimd.dma_start(
                    outq[jo * 128 : (jo + 1) * 128, qc * QN : (qc + 1) * QN], ot[:]
                )

        prev = None      # (kt, p_tiles, u_t, d_t) pending PV
        prev_epi = None  # (qc, u_t, d_t) pending normalize+projection
        pend_oproj = None
        for qc in range(NQC):
            u_t = d_t = None
            for kt in range(KT):
                p_tiles = emit_scores_exp(qc, kt)
                if prev is not None:
                    emit_pv(*prev)
                if kt == 0:
                    if prev_epi is not None:
                        # normalize on DVE now; out-proj matmuls deferred to
                        # kt==2 so they never block next-qc scores in the
                        # PE FIFO
                        pend_oproj = (prev_epi[0], emit_norm(*prev_epi))
                        prev_epi = None
                    u_t = [
                        pacc.tile([128, 512], F32, tag="u", name=f"u{qc}_{jo}")
                        for jo in range(2)
                    ]
                    d_t = [
                        pacc.tile([128, 512], F32, tag="u", name=f"d{qc}_{jo}")[
                            :, 0:QN
                        ]
                        for jo in range(2)
                    ]
                if kt == 2 and pend_oproj is not None:
                    emit_oproj(*pend_oproj)
                    pend_oproj = None
                prev = (kt, p_tiles, u_t, d_t)
            prev_epi = (qc, u_t, d_t)
        emit_pv(*prev)
        emit_oproj(prev_epi[0], emit_norm(*prev_epi))

    return nc


_CACHE: dict = {}


def _split_matmul_waits(nc: bass.Bass):
    """walrus's fp32r self-loading matmul (S3 LW struct) accepts only one
    sync-wait command; peel extra waits onto PE EventSemaphore ops inserted
    immediately before the matmul (same sync point, so no deadlock risk)."""
    import bass_rust

    n_new = 0
    for fn in nc.m.functions:
        for block in fn.blocks:
            insts = list(block.instructions)
            out = []
            changed = False
            skip = (
                mybir.InstEventSemaphore,
                mybir.InstAllEngineBarrier,
                mybir.InstHalt,
            )
            for inst in insts:
                if not isinstance(inst, skip) and inst.sync_info is not None:
                    si = inst.sync_info
                    waits = list(si.on_wait)
                    if len(waits) > 1:
                        for w in waits[:-1]:
                            ev = mybir.InstEventSemaphore(
                                name=f"WSPLIT-{n_new}", ins=[], outs=[]
                            )
                            ev.engine = inst.engine
                            ev.sync_info = bass_rust.SyncInfo(
                                on_wait=[w], on_update=[]
                            )
                            out.append(ev)
                            n_new += 1
                        inst.sync_info = bass_rust.SyncInfo(
                            on_wait=[waits[-1]], on_update=list(si.on_update)
                        )
                        changed = True
                out.append(inst)
            if changed:
                block.instructions = out
    return n_new


def get_nc() -> bass.Bass:
    if "nc" not in _CACHE:
        nc = bass.Bass()
        build_kernel(nc)
        _split_matmul_waits(nc)
        nc.finalize()
        _CACHE["nc"] = nc
    return _CACHE["nc"]


def make_core_inputs(feat, src, Wq, bq, Wk, bk, Wv, bv, Wo, bo):
    """Host-side sharding / layout prep. Returns list of 8 input dicts."""
    import ml_dtypes

    f32 = np.float32
    bf16 = ml_dtypes.bfloat16
    feat = np.asarray(feat, f32)
    src = np.asarray(src, f32)
    Wq, Wk, Wv, Wo = (np.asarray(x, f32) for x in (Wq, Wk, Wv, Wo))
    bq, bk, bv, bo = (np.asarray(x, f32) for x in (bq, bk, bv, bo))

    wqt = np.ascontiguousarray(Wq.T.reshape(2, 128, C).transpose(1, 0, 2))
    wot = np.ascontiguousarray(Wo.T.reshape(2, 128, C).transpose(1, 0, 2))

    # conv-tap layouts: wkc[kh*29+c, kw, cout] = Wk[cout, 9c+3kh+kw] (0 pad)
    wkc = np.zeros((128, 3, C), f32)
    wvc = np.zeros((128, 3, C), f32)
    for kh in range(3):
        for kw in range(3):
            for c in range(CF):
                j = 9 * c + 3 * kh + kw
                if j < C:
                    wkc[kh * CF + c, kw, :] = Wk[:, j]
                    wvc[kh * CF + c, kw, :] = Wv[:, j]
    wkc = wkc.astype(bf16)
    wvc = wvc.astype(bf16)
    onesd = np.ones((128, 32), bf16)

    bq2 = np.ascontiguousarray(bq.reshape(2, 128).T)
    bk2 = np.ascontiguousarray(bk.reshape(2, 128).T)
    boev = Wo @ bv + bo
    boe = np.ascontiguousarray(boev.reshape(2, 128).T)

    shared = dict(
        wqt=wqt, wot=wot, wkc=wkc, wvc=wvc, bq2=bq2, bk2=bk2, boe=boe, onesd=onesd
    )

    # featv[kh*29+c, kw, oh*64+ow] = feat[b, c, 2oh+kh-1, 2ow+kw-1], 0 OOB
    featv_b = []
    for b in range(B):
        fp = np.pad(feat[b, :CF], ((0, 0), (1, 1), (1, 1)))
        fv = np.zeros((128, 3, L), f32)
        for kh in range(3):
            for kw in range(3):
                patch = fp[:, kh : kh + 2 * H : 2, kw : kw + 2 * W : 2]
                fv[kh * CF : kh * CF + CF, kw, :] = patch.reshape(CF, L)
        featv_b.append(fv.astype(bf16))

    in_maps = []
    for core in range(NCORE):
        b, qi = divmod(core, 4)
        m = dict(shared)
        m["featv"] = featv_b[b]
        sl = src[b].reshape(C, L)[:, qi * QCHUNK : (qi + 1) * QCHUNK]
        m["srcq"] = np.ascontiguousarray(
            sl.reshape(2, 128, QCHUNK).transpose(1, 0, 2)
        )
        in_maps.append(m)
    return in_maps


def _ensure_ntff_hook():
    """Provide antenv.axon_hooks if the image lacks it (needed for trace=True)."""
    import contextlib
    import ctypes
    import os
    import sys
    import types

    try:
        import antenv.axon_hooks  # noqa: F401

        return
    except ImportError:
        pass

    mod = types.ModuleType("antenv.axon_hooks")
    box = [None]
    mod.set_axon_ntff_profile_hook = lambda h: box.__setitem__(0, h)
    mod.get_axon_ntff_profile_hook = lambda: box[0]
    sys.modules["antenv.axon_hooks"] = mod
    import antenv

    antenv.axon_hooks = mod

    so_path = os.environ.get("PJRT_LIBRARY_PATH", "/opt/axon/libaxon_pjrt.so")
    try:
        lib = ctypes.CDLL(so_path)
    except OSError:
        return
    if not hasattr(lib, "axon_start_nrt_profile"):
        return
    lib.axon_start_nrt_profile.argtypes = [
        ctypes.POINTER(ctypes.c_int64),
        ctypes.c_size_t,
    ]
    lib.axon_start_nrt_profile.restype = ctypes.c_int64
    lib.axon_stop_nrt_profile.argtypes = [ctypes.c_char_p]
    lib.axon_stop_nrt_profile.restype = ctypes.c_int64

    @contextlib.contextmanager
    def _hook(output_dir, device_ids):
        import jax

        jax.devices()
        if device_ids:
            ids = (ctypes.c_int64 * len(device_ids))(*device_ids)
            rc = lib.axon_start_nrt_profile(ids, len(device_ids))
        else:
            rc = lib.axon_start_nrt_profile(None, 0)
        if rc != 0:
            raise RuntimeError(f"axon_start_nrt_profile rc={rc}")
        try:
            yield
        finally:
            n = lib.axon_stop_nrt_profile(str(output_dir).encode())
            print(f"profile: {n} file(s) written to {output_dir}", file=sys.stderr)

    box[0] = _hook


def run(inputs: dict, trace: bool = False, trace_cores=None):
    _ensure_ntff_hook()
    from concourse.bass_utils import run_bass_kernel_spmd

    nc = get_nc()
    in_maps = make_core_inputs(**inputs)
    res = run_bass_kernel_spmd(
        nc,
        in_maps,
        list(range(NCORE)),
        trace=trace,
        trace_cores=trace_cores,
    )
    out = np.empty((B, C, L), np.float32)
    for core in range(NCORE):
        b, qi = divmod(core, 4)
        out[b, :, qi * QCHUNK : (qi + 1) * QCHUNK] = res.results[core]["outq"]
    return out.reshape(B, C, H, W), res


def kernel(feat, src, Wq, bq, Wk, bk, Wv, bv, Wo, bo):
    out, _ = run(
        dict(feat=feat, src=src, Wq=Wq, bq=bq, Wk=Wk, bk=bk, Wv=Wv, bv=bv, Wo=Wo, bo=bo)
    )
    return out
